# revision 12
# baseline (speedup 1.0000x reference)
"""3-layer GAT (GATConv x3 + log_softmax) on 8 Trainium2 NeuronCores — V3.

Strategy (dst-sharded, edge-parallel within core). V3 replaces the per-edge-slot
indirect DMAs (994ns SWDGE overhead each, ~3000/layer) with InstDMAGatherAnt
whole-window gathers:

- Node tables are [NPAD, 128] fp16 (256B rows = h only). Per-edge a_src is
  recomputed on-device from gathered h (mult by as-blockdiag + reduce),
  which the 256B-row constraint of dma_gather forces anyway.
- dma_gather indices are int16 (max 32767), so tables are split in two
  25088-row halves; each window's edge slots are grouped [A-half | B-half].
- Indices live in SBUF wrapped [16, n/16] and replicated across all eight
  16-partition groups (each SWDGE queue's Q7 rx/tx pair reads its own
  32-partition window). 4 SWDGE queues parallelize descriptor generation.
- L3 redesigned: table3 = z = elu(out2) (so the gather row is full-width),
  aggregation happens on z, and W3 is applied post-aggregation per window.
- Messages reduced over k by a contiguous halving tree (no strided DVE pass).

Host-side prep: per-core contiguous dst ranges, degree-desc sort within core
so each 128-node window has uniform max degree; edges grouped by
(core, window, dst slot, half, k).
"""
import numpy as np

import concourse.bass as bass
import concourse.mybir as mybir
import concourse.tile as tile
from concourse.masks import make_identity

# ---- problem constants (hardcoded per contest rules) ----
N = 50000
E = 800000
F_IN = 300
HEADS = 4
PER_HEAD = 32
HID = 128
N_CLASSES = 9
NEG_SLOPE = 0.2

NC_ = 8
NPER = 6272          # nodes per core (49 * 128)
NPAD = NC_ * NPER    # 50176
HALF = NPAD // 2     # 25088 (< 32768 so int16 indices reach every row)
P = 128
NWIN = NPER // P     # 49

f32 = mybir.dt.float32
f16 = mybir.dt.float16
i16 = mybir.dt.int16
AF = mybir.ActivationFunctionType

TWA = 132  # augmented transform width: [W | W@ad_blockdiag]
import os as _os

NQ = int(_os.environ.get("GAT_NQ", "4"))  # SWDGE queues
DBG_SKIP = set(_os.environ.get("GAT_SKIP", "").split(","))
GW = 7     # windows per batched-tail group (NWIN = 7*7)


# ----------------------------------------------------------------------------
# host-side graph prep
# ----------------------------------------------------------------------------
def prep_graph(edge_index):
    s = np.asarray(edge_index[0], dtype=np.int64)
    d = np.asarray(edge_index[1], dtype=np.int64)
    deg = np.bincount(d, minlength=N)

    old_of_new = np.full(NPAD, -1, dtype=np.int64)
    new_of_old = np.full(N, -1, dtype=np.int64)
    bounds = [min(c * NPER, N) for c in range(NC_ + 1)]
    for c in range(NC_):
        lo, hi = bounds[c], bounds[c + 1]
        nodes = np.arange(lo, hi)
        order = nodes[np.argsort(-deg[nodes], kind="stable")]
        old_of_new[c * NPER : c * NPER + len(order)] = order
        new_of_old[order] = c * NPER + np.arange(len(order))

    s_new = new_of_old[s]
    d_new = new_of_old[d]

    order_all = np.lexsort((s_new, d_new))
    s_sorted = s_new[order_all]
    d_sorted = d_new[order_all]
    starts = np.searchsorted(d_sorted, np.arange(NPAD))
    ends = np.searchsorted(d_sorted, np.arange(NPAD) + 1)

    # per (core, window, slot): src lists split by table half
    A = {}  # (c, w) -> [P] list of arrays (src < HALF)
    B = {}  # (c, w) -> [P] list of arrays (src >= HALF, rebased)
    K1 = np.zeros((NC_, NWIN), np.int64)
    K2 = np.zeros((NC_, NWIN), np.int64)
    for c in range(NC_):
        for w in range(NWIN):
            base = c * NPER + w * P
            la, lb = [], []
            for p in range(P):
                srcs = s_sorted[starts[base + p] : ends[base + p]]
                a = srcs[srcs < HALF]
                b = srcs[srcs >= HALF] - HALF
                la.append(a)
                lb.append(b)
            A[c, w] = la
            B[c, w] = lb
            K1[c, w] = max((len(a) for a in la), default=0)
            K2[c, w] = max((len(b) for b in lb), default=0)

    Ks1 = K1.max(axis=0)  # unified across cores (SPMD single program)
    Ks2 = K2.max(axis=0)
    return A, B, Ks1, Ks2, old_of_new


def wrap_idx(flat):
    """k-major flat slot indices [n] -> [128, n//16] int16, wrapped in 16
    partitions and replicated across the eight 16-partition groups."""
    n = len(flat)
    w = np.asarray(flat, np.int16).reshape(n // 16, 16).T  # [16, n//16]
    return np.tile(w, (8, 1))


def build_idx_mask(A, B, Ks1, Ks2):
    """Resident index/mask tensors. idx16: [NC_, 128, 8*S]; mask [NC_, P, S]."""
    S = int((Ks1 + Ks2).sum())
    idx16 = np.zeros((NC_, 128, 8 * S), np.int16)
    mask = np.zeros((NC_, P, S), np.float16)
    for c in range(NC_):
        col8 = 0
        col = 0
        for w in range(NWIN):
            k1, k2 = int(Ks1[w]), int(Ks2[w])
            if k1:
                arr = np.zeros((P, k1), np.int64)
                for p in range(P):
                    a = A[c, w][p]
                    arr[p, : len(a)] = a
                    mask[c, p, col : col + len(a)] = 1.0
                idx16[c, :, col8 : col8 + 8 * k1] = wrap_idx(arr.T.ravel())
                col8 += 8 * k1
                col += k1
            if k2:
                arr = np.zeros((P, k2), np.int64)
                for p in range(P):
                    b = B[c, w][p]
                    arr[p, : len(b)] = b
                    mask[c, p, col : col + len(b)] = 1.0
                idx16[c, :, col8 : col8 + 8 * k2] = wrap_idx(arr.T.ravel())
                col8 += 8 * k2
                col += k2
    return idx16, mask, S


# ----------------------------------------------------------------------------
# compile passes: library loads + walrus wait-split workaround
# ----------------------------------------------------------------------------
def insert_library_loads(nc):
    """Bacc.insert_library_loads equivalent for raw Bass: makes
    library-tracked Pool instructions (InstDMAGatherAnt) runnable."""
    from concourse.bacc import _bass_rust
    from concourse.library_config import all_libraries, standard

    inst_type_to_lib_mask = {}
    for lib in all_libraries:
        for inst_type in lib.instructions:
            inst_type_to_lib_mask[inst_type] = inst_type_to_lib_mask.get(
                inst_type, 0
            ) | (1 << lib.index)
    _bass_rust.insert_library_loads(
        nc, inst_type_to_lib_mask, len(all_libraries), standard.index
    )
    mybir.codegen_inst_isa_subclasses(nc)


def split_excess_waits(nc, max_waits=1):
    import copy

    n = 0
    for f in nc.m.functions:
        for blk in f.blocks:
            new_insts = []
            for ins in blk.instructions:
                need = (
                    ins.sync_info is not None and len(ins.sync_info.on_wait) > max_waits
                ) or (
                    isinstance(ins, mybir.InstDMACopy)
                    and getattr(ins, "queue", "") == "qPoolDynamic"
                    and ins.sync_info is not None
                    and len(ins.sync_info.on_wait) > 0
                )
                if need:
                    for w in list(ins.sync_info.on_wait):
                        noop = mybir.InstNoOp(
                            name=f"wait_split_{n}",
                            text_hint="wait_split",
                            bass_nofuse=True,
                        )
                        n += 1
                        noop.engine = ins.engine
                        si = copy.deepcopy(ins.sync_info)
                        si.on_update = type(si.on_update)()
                        si.on_wait = type(si.on_wait)([copy.deepcopy(w)])
                        noop.sync_info = si
                        new_insts.append(noop)
                    ins.sync_info.on_wait = type(ins.sync_info.on_wait)()
                new_insts.append(ins)
            if n:
                blk.instructions = new_insts
    return n


# ----------------------------------------------------------------------------
# device kernel builder
# ----------------------------------------------------------------------------
def build_nc(Ks1, Ks2, S, repeat=1, no_gather=False, nlayers=3):
    """One SPMD program; per-core data differs only in tensor contents."""
    nc = bass.Bass(num_swdge_queues=NQ)
    Ks = [int(a + b) for a, b in zip(Ks1, Ks2)]
    KMAX = max(Ks)

    xT = nc.declare_dram_parameter("xT", [F_IN, NPER], f16, isOutput=False)
    w1aug = nc.declare_dram_parameter("w1aug", [F_IN, TWA], f16, isOutput=False)
    w2aug = nc.declare_dram_parameter("w2aug", [HID, TWA], f16, isOutput=False)
    w3c = nc.declare_dram_parameter("w3c", [HID, N_CLASSES], f16, isOutput=False)
    # per-partition-replicated score vectors
    as1r = nc.declare_dram_parameter("as1r", [P, HID], f16, isOutput=False)
    as2r = nc.declare_dram_parameter("as2r", [P, HID], f16, isOutput=False)
    w3asr = nc.declare_dram_parameter("w3asr", [P, HID], f16, isOutput=False)
    w3adr = nc.declare_dram_parameter("w3adr", [P, HID], f16, isOutput=False)
    bias1 = nc.declare_dram_parameter("bias1", [P, HID], f32, isOutput=False)
    bias2 = nc.declare_dram_parameter("bias2", [P, HID], f32, isOutput=False)
    bias3 = nc.declare_dram_parameter("bias3", [P, N_CLASSES], f32, isOutput=False)
    idx_in = nc.declare_dram_parameter("idx", [128, 8 * S], i16, isOutput=False)
    mask_in = nc.declare_dram_parameter("mask", [P, S], f16, isOutput=False)
    out_ext = nc.declare_dram_parameter("out", [NPER, N_CLASSES], f32, isOutput=True)

    shard = [nc.dram_tensor(f"shard{l}", [NPER, HID], f16) for l in range(3)]
    table_sh = [
        nc.dram_tensor(f"table_sh{l}", [NPAD, HID], f16, addr_space="Shared")
        for l in range(3)
    ]

    rg = [list(range(NC_))]

    with tile.TileContext(nc) as tc:
        with (
            tc.tile_pool(name="resident", bufs=1) as rp,
            tc.tile_pool(name="sbuf", bufs=3) as pool,
            tc.tile_pool(name="gp", bufs=3) as gpool,
            tc.tile_pool(name="mp", bufs=2) as mpool,
            tc.tile_pool(name="aggp", bufs=2) as agpool,
            tc.tile_pool(name="psum", bufs=2, space="PSUM") as pp,
            tc.tile_pool(name="psum_t", bufs=2, space="PSUM") as ppt,
        ):
            # ---------- residents ----------
            idx_sb = rp.tile([128, 8 * S], i16)
            nc.sync.dma_start(out=idx_sb[:], in_=idx_in[:])
            mask_sb = rp.tile([P, S], f16)
            nc.sync.dma_start(out=mask_sb[:], in_=mask_in[:])
            ident = rp.tile([P, P], f32)
            make_identity(nc, ident[:])
            b1_sb = rp.tile([P, HID], f32)
            nc.sync.dma_start(out=b1_sb[:], in_=bias1[:])
            b2_sb = rp.tile([P, HID], f32)
            nc.sync.dma_start(out=b2_sb[:], in_=bias2[:])
            b3_sb = rp.tile([P, N_CLASSES], f32)
            nc.sync.dma_start(out=b3_sb[:], in_=bias3[:])
            w2aug_sb = rp.tile([P, TWA], f16)
            nc.sync.dma_start(out=w2aug_sb[:], in_=w2aug[:])
            w3_sb = rp.tile([P, N_CLASSES], f16)
            nc.sync.dma_start(out=w3_sb[:], in_=w3c[:])
            asr_sb = [rp.tile([P, HID], f16, name=f"asv{l}") for l in range(3)]
            nc.sync.dma_start(out=asr_sb[0][:], in_=as1r[:])
            nc.sync.dma_start(out=asr_sb[1][:], in_=as2r[:])
            nc.sync.dma_start(out=asr_sb[2][:], in_=w3asr[:])
            w3ad_sb = rp.tile([P, HID], f16)
            nc.sync.dma_start(out=w3ad_sb[:], in_=w3adr[:])
            w1_sb = rp.tile([P, 3 * TWA], f16)  # 3 k-chunks of w1aug
            for kc in range(3):
                kd = min(P, F_IN - kc * P)
                nc.sync.dma_start(
                    out=w1_sb[:kd, kc * TWA : kc * TWA + TWA],
                    in_=w1aug[kc * P : kc * P + kd, :],
                )
            # per-layer a_dst of own nodes; L1/L2: 4/window, L3: 1/window
            adst = [rp.tile([P, NWIN * 4], f32, name=f"adst{l}") for l in range(2)]
            adst3 = rp.tile([P, NWIN], f32)

            regs = {}

            def reg_of(v):
                if v not in regs:
                    regs[v] = nc.gpsimd.to_reg(v)
                return regs[v]

            qctr = [0]

            def next_q():
                q = qctr[0] % NQ
                qctr[0] += 1
                return q

            for _rep in range(repeat):
                # ---------- layer-1 node transform ----------
                for t in range(NWIN):
                    hpsum = pp.tile([P, TWA], f32, tag="hpsum")
                    for kc in range(3):
                        kd = min(P, F_IN - kc * P)
                        xt = pool.tile([P, P], f16, tag="xt")
                        nc.sync.dma_start(
                            out=xt[:kd, :],
                            in_=xT[kc * P : kc * P + kd, t * P : (t + 1) * P],
                        )
                        nc.tensor.matmul(
                            out=hpsum[:],
                            lhsT=xt[:kd, :],
                            rhs=w1_sb[:kd, kc * TWA : kc * TWA + TWA],
                            start=(kc == 0),
                            stop=(kc == 2),
                        )
                    hrow = pool.tile([P, HID], f16, tag="hrow")
                    nc.vector.tensor_copy(out=hrow[:], in_=hpsum[:, 0:HID])
                    nc.vector.tensor_copy(
                        out=adst[0][:, t * 4 : (t + 1) * 4], in_=hpsum[:, HID : HID + 4]
                    )
                    nc.sync.dma_start(out=shard[0][t * P : (t + 1) * P, :], in_=hrow[:])

                # ---------- per-layer: allgather + edge phase ----------
                for l in range(nlayers):
                    nc.gpsimd.collective_compute(
                        "AllGather",
                        mybir.AluOpType.bypass,
                        ins=[shard[l][:]],
                        outs=[table_sh[l][:]],
                        replica_groups=rg,
                    )
                    tabA = table_sh[l][0:HALF, :]
                    tabB = table_sh[l][HALF:NPAD, :]
                    off = 0   # mask / k-slot column offset
                    off8 = 0  # idx16 column offset
                    agg_g = None
                    for w in range(NWIN):
                        gi = w % GW
                        if gi == 0:
                            agg_g = agpool.tile([P, GW, HID], f32, tag="agg_g")
                        k1, k2 = int(Ks1[w]), int(Ks2[w])
                        K = k1 + k2
                        g = gpool.tile([P, KMAX, HID], f16, tag="g")
                        if no_gather:
                            nc.vector.memset(g[:, 0, :], 0.125)
                        else:
                            # SWDGE ring carveout limits one gather to ~64
                            # descs/engine -> chunk to <= 8 k-slots (1024 idxs)
                            CH = 8
                            for base, cnt, tab in (
                                (0, k1, tabA),
                                (k1, k2, tabB),
                            ):
                                for j in range(0, cnt, CH):
                                    ck = min(CH, cnt - j)
                                    nc.gpsimd.dma_gather(
                                        out_ap=g[:, base + j : base + j + ck, :],
                                        in_ap=tab,
                                        idxs_ap=idx_sb[:, off8 : off8 + 8 * ck],
                                        num_idxs=128 * ck,
                                        num_idxs_reg=reg_of(128 * ck),
                                        elem_size=HID,
                                        queue_num=next_q(),
                                    )
                                    off8 += 8 * ck

                        # ---- per-edge a_src from gathered h ----
                        t_as = mpool.tile([P, KMAX, HID], f16, tag="t_as")
                        nc.vector.tensor_tensor(
                            out=t_as[:, :K, :],
                            in0=g[:, :K, :],
                            in1=asr_sb[l][:].unsqueeze(1).to_broadcast([P, K, HID]),
                            op=mybir.AluOpType.mult,
                        )
                        if l < 2:
                            asrc = pool.tile([P, KMAX, 4], f32, tag="asrc")
                            nc.vector.tensor_reduce(
                                out=asrc[:, :K, :].rearrange("p k h -> p (k h)"),
                                in_=t_as[:, :K, :].rearrange(
                                    "p k (h c) -> p (k h) c", h=4
                                ),
                                axis=mybir.AxisListType.X,
                                op=mybir.AluOpType.add,
                            )
                            # ---- scores ----
                            lr = pool.tile([P, KMAX, 4], f32, tag="lr")
                            nc.vector.tensor_tensor(
                                out=lr[:, :K, :],
                                in0=asrc[:, :K, :],
                                in1=adst[l][:, w * 4 : (w + 1) * 4]
                                .unsqueeze(1)
                                .to_broadcast([P, K, 4]),
                                op=mybir.AluOpType.add,
                            )
                            nc.vector.scalar_tensor_tensor(
                                out=lr[:, :K, :],
                                in0=lr[:, :K, :],
                                scalar=NEG_SLOPE,
                                in1=lr[:, :K, :],
                                op0=mybir.AluOpType.mult,
                                op1=mybir.AluOpType.max,
                            )
                            e = pool.tile([P, KMAX, 4], f32, tag="e")
                            nc.scalar.activation(
                                out=e[:, :K, :], in_=lr[:, :K, :], func=AF.Exp
                            )
                            nc.vector.tensor_tensor(
                                out=e[:, :K, :],
                                in0=e[:, :K, :],
                                in1=mask_sb[:, off : off + K]
                                .unsqueeze(2)
                                .to_broadcast([P, K, 4]),
                                op=mybir.AluOpType.mult,
                            )
                            den = pool.tile([P, 4], f32, tag="den")
                            nc.vector.tensor_reduce(
                                out=den[:, :],
                                in_=e[:, :K, :].transpose([0, 2, 1]),
                                axis=mybir.AxisListType.X,
                                op=mybir.AluOpType.add,
                            )
                            nc.vector.tensor_scalar_add(
                                out=den[:, :], in0=den[:, :], scalar1=1e-30
                            )
                            rden = pool.tile([P, 4], f32, tag="rden")
                            nc.vector.reciprocal(out=rden[:, :], in_=den[:, :])
                            alpha = pool.tile([P, KMAX, 4], f32, tag="alpha")
                            nc.vector.tensor_tensor(
                                out=alpha[:, :K, :],
                                in0=e[:, :K, :],
                                in1=rden[:, :].unsqueeze(1).to_broadcast([P, K, 4]),
                                op=mybir.AluOpType.mult,
                            )
                            # ---- messages (contiguous mult + halving tree) ----
                            m = mpool.tile([P, KMAX, HID], f16, tag="m")
                            nc.vector.tensor_tensor(
                                out=m[:, :K, :].rearrange("p k (h c) -> p k h c", h=4),
                                in0=g[:, :K, :].rearrange("p k (h c) -> p k h c", h=4),
                                in1=alpha[:, :K, :]
                                .unsqueeze(3)
                                .to_broadcast([P, K, 4, PER_HEAD]),
                                op=mybir.AluOpType.mult,
                            )
                            cur = K
                            while cur > 1:
                                half = cur // 2
                                nc.vector.tensor_tensor(
                                    out=m[:, :half, :],
                                    in0=m[:, :half, :],
                                    in1=m[:, cur - half : cur, :],
                                    op=mybir.AluOpType.add,
                                )
                                cur -= half
                            nc.vector.tensor_copy(
                                out=agg_g[:, gi, :], in_=m[:, 0, :]
                            )
                        else:
                            # single-head L3 on z rows
                            asrc3 = pool.tile([P, KMAX], f32, tag="asrc3")
                            nc.vector.tensor_reduce(
                                out=asrc3[:, :K],
                                in_=t_as[:, :K, :],
                                axis=mybir.AxisListType.X,
                                op=mybir.AluOpType.add,
                            )
                            lr3 = pool.tile([P, KMAX], f32, tag="lr3")
                            nc.vector.tensor_tensor(
                                out=lr3[:, :K],
                                in0=asrc3[:, :K],
                                in1=adst3[:, w : w + 1].to_broadcast([P, K]),
                                op=mybir.AluOpType.add,
                            )
                            nc.vector.scalar_tensor_tensor(
                                out=lr3[:, :K],
                                in0=lr3[:, :K],
                                scalar=NEG_SLOPE,
                                in1=lr3[:, :K],
                                op0=mybir.AluOpType.mult,
                                op1=mybir.AluOpType.max,
                            )
                            e3 = pool.tile([P, KMAX], f32, tag="e3")
                            nc.scalar.activation(
                                out=e3[:, :K], in_=lr3[:, :K], func=AF.Exp
                            )
                            nc.vector.tensor_tensor(
                                out=e3[:, :K],
                                in0=e3[:, :K],
                                in1=mask_sb[:, off : off + K],
                                op=mybir.AluOpType.mult,
                            )
                            den3 = pool.tile([P, 1], f32, tag="den3")
                            nc.vector.tensor_reduce(
                                out=den3[:, :],
                                in_=e3[:, :K],
                                axis=mybir.AxisListType.X,
                                op=mybir.AluOpType.add,
                            )
                            nc.vector.tensor_scalar_add(
                                out=den3[:, :], in0=den3[:, :], scalar1=1e-30
                            )
                            rden3 = pool.tile([P, 1], f32, tag="rden3")
                            nc.vector.reciprocal(out=rden3[:, :], in_=den3[:, :])
                            m = mpool.tile([P, KMAX, HID], f16, tag="m")
                            nc.vector.tensor_tensor(
                                out=m[:, :K, :],
                                in0=g[:, :K, :],
                                in1=e3[:, :K].unsqueeze(2).to_broadcast([P, K, HID]),
                                op=mybir.AluOpType.mult,
                            )
                            cur = K
                            while cur > 1:
                                half = cur // 2
                                nc.vector.tensor_tensor(
                                    out=m[:, :half, :],
                                    in0=m[:, :half, :],
                                    in1=m[:, cur - half : cur, :],
                                    op=mybir.AluOpType.add,
                                )
                                cur -= half
                            nc.vector.tensor_tensor(
                                out=agg_g[:, gi, :],
                                in0=m[:, 0, :],
                                in1=rden3[:, 0:1].to_broadcast([P, HID]),
                                op=mybir.AluOpType.mult,
                            )
                        off += K

                        # ---- batched tail once per group of GW windows ----
                        if gi == GW - 1:
                            w0 = w - (GW - 1)
                            if l < 2:
                                bsb = b1_sb if l == 0 else b2_sb
                                y = pool.tile([P, GW, HID], f32, tag="y_g")
                                nc.vector.tensor_tensor(
                                    out=y[:],
                                    in0=agg_g[:],
                                    in1=bsb[:].unsqueeze(1).to_broadcast([P, GW, HID]),
                                    op=mybir.AluOpType.add,
                                )
                                neg = pool.tile([P, GW, HID], f32, tag="neg_g")
                                nc.vector.tensor_scalar_min(
                                    out=neg[:], in0=y[:], scalar1=0.0
                                )
                                en = pool.tile([P, GW, HID], f32, tag="en_g")
                                nc.scalar.activation(out=en[:], in_=neg[:], func=AF.Exp)
                                elu = pool.tile([P, GW, HID], f32, tag="elu_g")
                                nc.vector.scalar_tensor_tensor(
                                    out=elu[:],
                                    in0=y[:],
                                    scalar=0.0,
                                    in1=en[:],
                                    op0=mybir.AluOpType.max,
                                    op1=mybir.AluOpType.add,
                                )
                                nc.vector.tensor_scalar_add(
                                    out=elu[:], in0=elu[:], scalar1=-1.0
                                )
                                for i in range(GW):
                                    ww = w0 + i
                                    if l == 0:
                                        eluT_p = ppt.tile([P, P], f32, tag="eluT_p")
                                        nc.tensor.transpose(
                                            out=eluT_p[:],
                                            in_=elu[:, i, :],
                                            identity=ident[:],
                                        )
                                        eluT = pool.tile([P, P], f16, tag="eluT")
                                        nc.vector.tensor_copy(
                                            out=eluT[:], in_=eluT_p[:]
                                        )
                                        h2psum = pp.tile([P, TWA], f32, tag="hpsum")
                                        nc.tensor.matmul(
                                            out=h2psum[:],
                                            lhsT=eluT[:],
                                            rhs=w2aug_sb[:],
                                            start=True,
                                            stop=True,
                                        )
                                        srow = pool.tile([P, HID], f16, tag="srow")
                                        nc.vector.tensor_copy(
                                            out=srow[:], in_=h2psum[:, 0:HID]
                                        )
                                        nc.vector.tensor_copy(
                                            out=adst[1][:, ww * 4 : ww * 4 + 4],
                                            in_=h2psum[:, HID : HID + 4],
                                        )
                                        nc.sync.dma_start(
                                            out=shard[1][ww * P : (ww + 1) * P, :],
                                            in_=srow[:],
                                        )
                                    else:
                                        # table3 = z = elu(out2); adst3 = z @ w3ad
                                        zrow = pool.tile([P, HID], f16, tag="zrow")
                                        nc.vector.tensor_copy(
                                            out=zrow[:], in_=elu[:, i, :]
                                        )
                                        scr = pool.tile([P, HID], f32, tag="scr")
                                        nc.vector.tensor_tensor(
                                            out=scr[:],
                                            in0=elu[:, i, :],
                                            in1=w3ad_sb[:],
                                            op=mybir.AluOpType.mult,
                                        )
                                        nc.vector.tensor_reduce(
                                            out=adst3[:, ww : ww + 1],
                                            in_=scr[:],
                                            axis=mybir.AxisListType.X,
                                            op=mybir.AluOpType.add,
                                        )
                                        nc.sync.dma_start(
                                            out=shard[2][ww * P : (ww + 1) * P, :],
                                            in_=zrow[:],
                                        )
                            else:
                                # L3 tail: per window W3 matmul, then batched
                                # bias+elu+log_softmax over GW windows
                                o3 = pool.tile([P, GW, N_CLASSES], f32, tag="o3g")
                                for i in range(GW):
                                    zT_p = ppt.tile([P, P], f32, tag="zT_p")
                                    nc.tensor.transpose(
                                        out=zT_p[:],
                                        in_=agg_g[:, i, :],
                                        identity=ident[:],
                                    )
                                    zT = pool.tile([P, P], f16, tag="zT")
                                    nc.vector.tensor_copy(out=zT[:], in_=zT_p[:])
                                    opsum = ppt.tile([P, N_CLASSES], f32, tag="opsum")
                                    nc.tensor.matmul(
                                        out=opsum[:],
                                        lhsT=zT[:],
                                        rhs=w3_sb[:],
                                        start=True,
                                        stop=True,
                                    )
                                    nc.vector.tensor_copy(
                                        out=o3[:, i, :], in_=opsum[:]
                                    )
                                y0 = pool.tile([P, GW, N_CLASSES], f32, tag="y90g")
                                nc.vector.tensor_tensor(
                                    out=y0[:],
                                    in0=o3[:],
                                    in1=b3_sb[:]
                                    .unsqueeze(1)
                                    .to_broadcast([P, GW, N_CLASSES]),
                                    op=mybir.AluOpType.add,
                                )
                                n9 = pool.tile([P, GW, N_CLASSES], f32, tag="n9g")
                                nc.vector.tensor_scalar_min(
                                    out=n9[:], in0=y0[:], scalar1=0.0
                                )
                                nc.scalar.activation(out=n9[:], in_=n9[:], func=AF.Exp)
                                yb = pool.tile([P, GW, N_CLASSES], f32, tag="y9g")
                                nc.vector.scalar_tensor_tensor(
                                    out=yb[:],
                                    in0=y0[:],
                                    scalar=0.0,
                                    in1=n9[:],
                                    op0=mybir.AluOpType.max,
                                    op1=mybir.AluOpType.add,
                                )
                                nc.vector.tensor_scalar_add(
                                    out=yb[:], in0=yb[:], scalar1=-1.0
                                )
                                e9 = pool.tile([P, GW, N_CLASSES], f32, tag="e9g")
                                nc.scalar.activation(out=e9[:], in_=yb[:], func=AF.Exp)
                                s9 = pool.tile([P, GW], f32, tag="s9g")
                                nc.vector.reduce_sum(
                                    out=s9[:], in_=e9[:], axis=mybir.AxisListType.X
                                )
                                l9 = pool.tile([P, GW], f32, tag="l9g")
                                nc.scalar.activation(out=l9[:], in_=s9[:], func=AF.Ln)
                                o9 = pool.tile([P, GW, N_CLASSES], f32, tag="o9g")
                                nc.vector.tensor_tensor(
                                    out=o9[:],
                                    in0=yb[:],
                                    in1=l9[:]
                                    .unsqueeze(2)
                                    .to_broadcast([P, GW, N_CLASSES]),
                                    op=mybir.AluOpType.subtract,
                                )
                                for i in range(GW):
                                    ww = w0 + i
                                    nc.sync.dma_start(
                                        out=out_ext[ww * P : (ww + 1) * P, :],
                                        in_=o9[:, i, :],
                                    )

    return nc


# ----------------------------------------------------------------------------
# host wrapper
# ----------------------------------------------------------------------------
def _np(x):
    return np.asarray(x)


def make_in_maps(inputs):
    x = _np(inputs["x"]).astype(np.float32)
    edge_index = _np(inputs["edge_index"])
    W1 = _np(inputs["W1"]).astype(np.float32)
    as1 = _np(inputs["as1"]).astype(np.float32)
    ad1 = _np(inputs["ad1"]).astype(np.float32)
    b1 = _np(inputs["b1"]).astype(np.float32)
    W2 = _np(inputs["W2"]).astype(np.float32)
    as2 = _np(inputs["as2"]).astype(np.float32)
    ad2 = _np(inputs["ad2"]).astype(np.float32)
    b2 = _np(inputs["b2"]).astype(np.float32)
    W3 = _np(inputs["W3"]).astype(np.float32)
    as3 = _np(inputs["as3"]).astype(np.float32)
    ad3 = _np(inputs["ad3"]).astype(np.float32)
    b3 = _np(inputs["b3"]).astype(np.float32)

    A, B, Ks1, Ks2, old_of_new = prep_graph(edge_index)
    idx16, mask, S = build_idx_mask(A, B, Ks1, Ks2)

    def blockdiag(a):  # [H, C] -> [H*C, H]
        H, C = a.shape
        out = np.zeros((H * C, H), np.float32)
        for h in range(H):
            out[h * C : (h + 1) * C, h] = a[h]
        return out

    def flat_as(a):  # [H, C] -> [H*C] (value at h*C+c = a[h, c])
        return a.reshape(-1)

    ad1b, ad2b = blockdiag(ad1), blockdiag(ad2)
    w1aug = np.concatenate([W1, W1 @ ad1b], axis=1).astype(np.float16)
    w2aug = np.concatenate([W2, W2 @ ad2b], axis=1).astype(np.float16)
    w3c = W3.astype(np.float16)
    as1r = np.broadcast_to(flat_as(as1), (P, HID)).astype(np.float16).copy()
    as2r = np.broadcast_to(flat_as(as2), (P, HID)).astype(np.float16).copy()
    w3asr = np.broadcast_to(W3 @ as3[0], (P, HID)).astype(np.float16).copy()
    w3adr = np.broadcast_to(W3 @ ad3[0], (P, HID)).astype(np.float16).copy()

    xT = np.zeros((F_IN, NPAD), np.float16)
    real = old_of_new >= 0
    xT[:, real] = x[old_of_new[real]].T.astype(np.float16)

    b1_bc = np.broadcast_to(b1, (P, HID)).copy()
    b2_bc = np.broadcast_to(b2, (P, HID)).copy()
    b3_bc = np.broadcast_to(b3, (P, N_CLASSES)).copy()

    in_maps = []
    for c in range(NC_):
        in_maps.append(
            {
                "xT": np.ascontiguousarray(xT[:, c * NPER : (c + 1) * NPER]),
                "w1aug": w1aug,
                "w2aug": w2aug,
                "w3c": w3c,
                "as1r": as1r,
                "as2r": as2r,
                "w3asr": w3asr,
                "w3adr": w3adr,
                "bias1": b1_bc,
                "bias2": b2_bc,
                "bias3": b3_bc,
                "idx": idx16[c],
                "mask": mask[c],
            }
        )
    return in_maps, Ks1, Ks2, S, old_of_new


def finalize_nc(nc):
    insert_library_loads(nc)
    split_excess_waits(nc)
    return nc


def kernel(**inputs):
    from concourse.bass_utils import run_bass_kernel_spmd

    in_maps, Ks1, Ks2, S, old_of_new = make_in_maps(inputs)
    nc = build_nc(Ks1, Ks2, S)
    finalize_nc(nc)
    res = run_bass_kernel_spmd(nc, in_maps, list(range(NC_)))
    out = np.zeros((N, N_CLASSES), np.float32)
    for c in range(NC_):
        rows = old_of_new[c * NPER : (c + 1) * NPER]
        m = rows >= 0
        out[rows[m]] = res.results[c]["out"][m]
    return out


# revision 26
# speedup vs baseline: 1.0951x; 1.0951x over previous
"""3-layer GAT (GATConv x3 + log_softmax) on 8 Trainium2 NeuronCores — V3.

Strategy (dst-sharded, edge-parallel within core). V3 replaces the per-edge-slot
indirect DMAs (994ns SWDGE overhead each, ~3000/layer) with InstDMAGatherAnt
whole-window gathers:

- Node tables are [NPAD, 128] fp16 (256B rows = h only). Per-edge a_src is
  recomputed on-device from gathered h (mult by as-blockdiag + reduce),
  which the 256B-row constraint of dma_gather forces anyway.
- dma_gather indices are int16 (max 32767), so tables are split in two
  25088-row halves; each window's edge slots are grouped [A-half | B-half].
- Indices live in SBUF wrapped [16, n/16] and replicated across all eight
  16-partition groups (each SWDGE queue's Q7 rx/tx pair reads its own
  32-partition window). 4 SWDGE queues parallelize descriptor generation.
- L3 redesigned: table3 = z = elu(out2) (so the gather row is full-width),
  aggregation happens on z, and W3 is applied post-aggregation per window.
- Messages reduced over k by a contiguous halving tree (no strided DVE pass).

Host-side prep: per-core contiguous dst ranges, degree-desc sort within core
so each 128-node window has uniform max degree; edges grouped by
(core, window, dst slot, half, k).
"""
import numpy as np

import concourse.bass as bass
import concourse.mybir as mybir
import concourse.tile as tile
from concourse.masks import make_identity

# ---- problem constants (hardcoded per contest rules) ----
N = 50000
E = 800000
F_IN = 300
HEADS = 4
PER_HEAD = 32
HID = 128
N_CLASSES = 9
NEG_SLOPE = 0.2

NC_ = 8
NPER = 6272          # nodes per core (49 * 128)
NPAD = NC_ * NPER    # 50176
HALF = NPAD // 2     # 25088 (< 32768 so int16 indices reach every row)
P = 128
NWIN = NPER // P     # 49

f32 = mybir.dt.float32
f16 = mybir.dt.float16
i16 = mybir.dt.int16
AF = mybir.ActivationFunctionType

TWA = 132  # augmented transform width: [W | W@ad_blockdiag]
import os as _os

NQ = int(_os.environ.get("GAT_NQ", "4"))  # SWDGE queues
DBG_SKIP = set(_os.environ.get("GAT_SKIP", "").split(","))
GW = 7     # windows per batched-tail group (NWIN = 7*7)


# ----------------------------------------------------------------------------
# host-side graph prep
# ----------------------------------------------------------------------------
def prep_graph(edge_index):
    s = np.asarray(edge_index[0], dtype=np.int64)
    d = np.asarray(edge_index[1], dtype=np.int64)
    deg = np.bincount(d, minlength=N)

    old_of_new = np.full(NPAD, -1, dtype=np.int64)
    new_of_old = np.full(N, -1, dtype=np.int64)
    bounds = [min(c * NPER, N) for c in range(NC_ + 1)]
    for c in range(NC_):
        lo, hi = bounds[c], bounds[c + 1]
        nodes = np.arange(lo, hi)
        order = nodes[np.argsort(-deg[nodes], kind="stable")]
        old_of_new[c * NPER : c * NPER + len(order)] = order
        new_of_old[order] = c * NPER + np.arange(len(order))

    s_new = new_of_old[s]
    d_new = new_of_old[d]

    order_all = np.lexsort((s_new, d_new))
    s_sorted = s_new[order_all]
    d_sorted = d_new[order_all]
    starts = np.searchsorted(d_sorted, np.arange(NPAD))
    ends = np.searchsorted(d_sorted, np.arange(NPAD) + 1)

    # per (core, window, slot): src lists split by table half
    A = {}  # (c, w) -> [P] list of arrays (src < HALF)
    B = {}  # (c, w) -> [P] list of arrays (src >= HALF, rebased)
    K1 = np.zeros((NC_, NWIN), np.int64)
    K2 = np.zeros((NC_, NWIN), np.int64)
    for c in range(NC_):
        for w in range(NWIN):
            base = c * NPER + w * P
            la, lb = [], []
            for p in range(P):
                srcs = s_sorted[starts[base + p] : ends[base + p]]
                a = srcs[srcs < HALF]
                b = srcs[srcs >= HALF] - HALF
                la.append(a)
                lb.append(b)
            A[c, w] = la
            B[c, w] = lb
            K1[c, w] = max((len(a) for a in la), default=0)
            K2[c, w] = max((len(b) for b in lb), default=0)

    Ks1 = K1.max(axis=0)  # unified across cores (SPMD single program)
    Ks2 = K2.max(axis=0)
    return A, B, Ks1, Ks2, old_of_new


def wrap_idx(flat):
    """k-major flat slot indices [n] -> [128, n//16] int16, wrapped in 16
    partitions and replicated across the eight 16-partition groups."""
    n = len(flat)
    w = np.asarray(flat, np.int16).reshape(n // 16, 16).T  # [16, n//16]
    return np.tile(w, (8, 1))


def build_idx_mask(A, B, Ks1, Ks2):
    """Resident index/mask tensors. idx16: [NC_, 128, 8*S]; mask [NC_, P, S]."""
    S = int((Ks1 + Ks2).sum())
    idx16 = np.zeros((NC_, 128, 8 * S), np.int16)
    mask = np.zeros((NC_, P, S), np.float16)
    for c in range(NC_):
        col8 = 0
        col = 0
        for w in range(NWIN):
            k1, k2 = int(Ks1[w]), int(Ks2[w])
            if k1:
                arr = np.zeros((P, k1), np.int64)
                for p in range(P):
                    a = A[c, w][p]
                    arr[p, : len(a)] = a
                    mask[c, p, col : col + len(a)] = 1.0
                idx16[c, :, col8 : col8 + 8 * k1] = wrap_idx(arr.T.ravel())
                col8 += 8 * k1
                col += k1
            if k2:
                arr = np.zeros((P, k2), np.int64)
                for p in range(P):
                    b = B[c, w][p]
                    arr[p, : len(b)] = b
                    mask[c, p, col : col + len(b)] = 1.0
                idx16[c, :, col8 : col8 + 8 * k2] = wrap_idx(arr.T.ravel())
                col8 += 8 * k2
                col += k2
    return idx16, mask, S


# ----------------------------------------------------------------------------
# compile passes: library loads + walrus wait-split workaround
# ----------------------------------------------------------------------------
def insert_library_loads(nc):
    """Bacc.insert_library_loads equivalent for raw Bass: makes
    library-tracked Pool instructions (InstDMAGatherAnt) runnable."""
    from concourse.bacc import _bass_rust
    from concourse.library_config import all_libraries, standard

    inst_type_to_lib_mask = {}
    for lib in all_libraries:
        for inst_type in lib.instructions:
            inst_type_to_lib_mask[inst_type] = inst_type_to_lib_mask.get(
                inst_type, 0
            ) | (1 << lib.index)
    _bass_rust.insert_library_loads(
        nc, inst_type_to_lib_mask, len(all_libraries), standard.index
    )
    mybir.codegen_inst_isa_subclasses(nc)


def split_excess_waits(nc, max_waits=1):
    import copy

    n = 0
    for f in nc.m.functions:
        for blk in f.blocks:
            new_insts = []
            for ins in blk.instructions:
                need = (
                    ins.sync_info is not None and len(ins.sync_info.on_wait) > max_waits
                ) or (
                    isinstance(ins, mybir.InstDMACopy)
                    and getattr(ins, "queue", "") == "qPoolDynamic"
                    and ins.sync_info is not None
                    and len(ins.sync_info.on_wait) > 0
                )
                if need:
                    for w in list(ins.sync_info.on_wait):
                        noop = mybir.InstNoOp(
                            name=f"wait_split_{n}",
                            text_hint="wait_split",
                            bass_nofuse=True,
                        )
                        n += 1
                        noop.engine = ins.engine
                        si = copy.deepcopy(ins.sync_info)
                        si.on_update = type(si.on_update)()
                        si.on_wait = type(si.on_wait)([copy.deepcopy(w)])
                        noop.sync_info = si
                        new_insts.append(noop)
                    ins.sync_info.on_wait = type(ins.sync_info.on_wait)()
                new_insts.append(ins)
            if n:
                blk.instructions = new_insts
    return n


# ----------------------------------------------------------------------------
# device kernel builder
# ----------------------------------------------------------------------------
def build_nc(Ks1, Ks2, S, repeat=1, no_gather=False, nlayers=3, local_ag=False):
    """One SPMD program; per-core data differs only in tensor contents."""
    nc = bass.Bass(num_swdge_queues=NQ)
    Ks = [int(a + b) for a, b in zip(Ks1, Ks2)]
    KMAX = max(Ks)

    xT = nc.declare_dram_parameter("xT", [F_IN, NPER], f16, isOutput=False)
    w1aug = nc.declare_dram_parameter("w1aug", [F_IN, TWA], f16, isOutput=False)
    w2aug = nc.declare_dram_parameter("w2aug", [HID, TWA], f16, isOutput=False)
    w3c = nc.declare_dram_parameter("w3c", [HID, N_CLASSES], f16, isOutput=False)
    # rotation-basis matrices: L1/L2 unrotation [128,128]; L3 [R3t|w3ad] [128,129]
    as1r = nc.declare_dram_parameter("as1r", [HID, HID], f16, isOutput=False)
    as2r = nc.declare_dram_parameter("as2r", [HID, HID], f16, isOutput=False)
    w3asr = nc.declare_dram_parameter("w3asr", [HID, HID + 1], f16, isOutput=False)
    bias1 = nc.declare_dram_parameter("bias1", [P, HID], f32, isOutput=False)
    bias2 = nc.declare_dram_parameter("bias2", [P, HID], f32, isOutput=False)
    bias3 = nc.declare_dram_parameter("bias3", [P, N_CLASSES], f32, isOutput=False)
    idx_in = nc.declare_dram_parameter("idx", [128, 8 * S], i16, isOutput=False)
    mask_in = nc.declare_dram_parameter("mask", [P, S], f16, isOutput=False)
    out_ext = nc.declare_dram_parameter("out", [NPER, N_CLASSES], f32, isOutput=True)

    shard = [nc.dram_tensor(f"shard{l}", [NPER, HID], f16) for l in range(3)]
    table_sh = [
        nc.dram_tensor(f"table_sh{l}", [NPAD, HID], f16, addr_space="Shared")
        for l in range(3)
    ]

    rg = [list(range(NC_))]

    with tile.TileContext(nc) as tc:
        with (
            tc.tile_pool(name="resident", bufs=1) as rp,
            tc.tile_pool(name="sbuf", bufs=3) as pool,
            tc.tile_pool(name="gp", bufs=4) as gpool,
            tc.tile_pool(name="mp", bufs=2) as mpool,
            tc.tile_pool(name="aggp", bufs=2) as agpool,
            tc.tile_pool(name="psum", bufs=2, space="PSUM") as pp,
            tc.tile_pool(name="psum_t", bufs=2, space="PSUM") as ppt,
        ):
            # ---------- residents ----------
            idx_sb = rp.tile([128, 8 * S], i16)
            nc.sync.dma_start(out=idx_sb[:], in_=idx_in[:])
            mask_sb = rp.tile([P, S], f16)
            nc.sync.dma_start(out=mask_sb[:], in_=mask_in[:])
            ident = rp.tile([P, P], f32)
            make_identity(nc, ident[:])
            b1_sb = rp.tile([P, HID], f32)
            nc.sync.dma_start(out=b1_sb[:], in_=bias1[:])
            b2_sb = rp.tile([P, HID], f32)
            nc.sync.dma_start(out=b2_sb[:], in_=bias2[:])
            b3_sb = rp.tile([P, N_CLASSES], f32)
            nc.sync.dma_start(out=b3_sb[:], in_=bias3[:])
            w2aug_sb = rp.tile([P, TWA], f16)
            nc.sync.dma_start(out=w2aug_sb[:], in_=w2aug[:])
            w3_sb = rp.tile([P, N_CLASSES], f16)
            nc.sync.dma_start(out=w3_sb[:], in_=w3c[:])
            # unrotation matrices (agg' -> agg) for L1/L2; L3 z' row builder
            unrot_sb = [rp.tile([P, HID], f16, name=f"unrot{l}") for l in range(2)]
            nc.sync.dma_start(out=unrot_sb[0][:], in_=as1r[:])
            nc.sync.dma_start(out=unrot_sb[1][:], in_=as2r[:])
            rhs3_sb = rp.tile([P, HID + 1], f16)  # [R3t | w3ad]
            nc.sync.dma_start(out=rhs3_sb[:], in_=w3asr[:])
            w1_sb = rp.tile([P, 3 * TWA], f16)  # 3 k-chunks of w1aug
            for kc in range(3):
                kd = min(P, F_IN - kc * P)
                nc.sync.dma_start(
                    out=w1_sb[:kd, kc * TWA : kc * TWA + TWA],
                    in_=w1aug[kc * P : kc * P + kd, :],
                )
            # per-layer a_dst of own nodes; L1/L2: 4/window, L3: 1/window
            adst = [rp.tile([P, NWIN * 4], f32, name=f"adst{l}") for l in range(2)]
            adst3 = rp.tile([P, NWIN], f32)

            regs = {}

            def reg_of(v):
                if v not in regs:
                    regs[v] = nc.gpsimd.to_reg(v)
                return regs[v]

            qctr = [0]

            def next_q():
                q = qctr[0] % NQ
                qctr[0] += 1
                return q

            for _rep in range(repeat):
                # ---------- layer-1 node transform ----------
                for t in range(NWIN):
                    hpsum = pp.tile([P, TWA], f32, tag="hpsum")
                    for kc in range(3):
                        kd = min(P, F_IN - kc * P)
                        xt = pool.tile([P, P], f16, tag="xt")
                        nc.sync.dma_start(
                            out=xt[:kd, :],
                            in_=xT[kc * P : kc * P + kd, t * P : (t + 1) * P],
                        )
                        nc.tensor.matmul(
                            out=hpsum[:],
                            lhsT=xt[:kd, :],
                            rhs=w1_sb[:kd, kc * TWA : kc * TWA + TWA],
                            start=(kc == 0),
                            stop=(kc == 2),
                        )
                    hrow = pool.tile([P, HID], f16, tag="hrow")
                    nc.vector.tensor_copy(out=hrow[:], in_=hpsum[:, 0:HID])
                    nc.vector.tensor_copy(
                        out=adst[0][:, t * 4 : (t + 1) * 4], in_=hpsum[:, HID : HID + 4]
                    )
                    nc.sync.dma_start(out=shard[0][t * P : (t + 1) * P, :], in_=hrow[:])

                # ---------- per-layer: allgather + edge phase ----------
                for l in range(nlayers):
                    if local_ag:
                        # single-core stand-in for TimelineSim (no collectives)
                        nc.sync.dma_start(
                            out=table_sh[l][0:NPER, :], in_=shard[l][:]
                        )
                    else:
                        nc.gpsimd.collective_compute(
                            "AllGather",
                            mybir.AluOpType.bypass,
                            ins=[shard[l][:]],
                            outs=[table_sh[l][:]],
                            replica_groups=rg,
                        )
                    tabA = table_sh[l][0:HALF, :]
                    tabB = table_sh[l][HALF:NPAD, :]
                    off = 0   # mask / k-slot column offset
                    off8 = 0  # idx16 column offset
                    agg_g = None
                    for w in range(NWIN):
                        gi = w % GW
                        if gi == 0:
                            agg_g = agpool.tile([P, GW, HID], f32, tag="agg_g")
                        k1, k2 = int(Ks1[w]), int(Ks2[w])
                        K = k1 + k2
                        g = gpool.tile([P, KMAX, HID], f16, tag="g")
                        if no_gather:
                            nc.vector.memset(g[:, 0, :], 0.125)
                        else:
                            # SWDGE ring carveout limits one gather to ~64
                            # descs/engine -> chunk to <= 8 k-slots (1024 idxs)
                            CH = 8
                            for base, cnt, tab in (
                                (0, k1, tabA),
                                (k1, k2, tabB),
                            ):
                                for j in range(0, cnt, CH):
                                    ck = min(CH, cnt - j)
                                    nc.gpsimd.dma_gather(
                                        out_ap=g[:, base + j : base + j + ck, :],
                                        in_ap=tab,
                                        idxs_ap=idx_sb[:, off8 : off8 + 8 * ck],
                                        num_idxs=128 * ck,
                                        num_idxs_reg=reg_of(128 * ck),
                                        elem_size=HID,
                                        queue_num=next_q(),
                                    )
                                    off8 += 8 * ck

                        # rotated table rows: a_src per head = component h*32
                        g4 = g[:, :K, :].rearrange("p k (h c) -> p k h c", h=4)
                        if l < 2:
                            # ---- scores ----
                            lr = pool.tile([P, KMAX, 4], f32, tag="lr")
                            nc.vector.tensor_tensor(
                                out=lr[:, :K, :],
                                in0=g4[:, :, :, 0],
                                in1=adst[l][:, w * 4 : (w + 1) * 4]
                                .unsqueeze(1)
                                .to_broadcast([P, K, 4]),
                                op=mybir.AluOpType.add,
                            )
                            nc.vector.scalar_tensor_tensor(
                                out=lr[:, :K, :],
                                in0=lr[:, :K, :],
                                scalar=NEG_SLOPE,
                                in1=lr[:, :K, :],
                                op0=mybir.AluOpType.mult,
                                op1=mybir.AluOpType.max,
                            )
                            e = pool.tile([P, KMAX, 4], f32, tag="e")
                            nc.scalar.activation(
                                out=e[:, :K, :], in_=lr[:, :K, :], func=AF.Exp
                            )
                            nc.vector.tensor_tensor(
                                out=e[:, :K, :],
                                in0=e[:, :K, :],
                                in1=mask_sb[:, off : off + K]
                                .unsqueeze(2)
                                .to_broadcast([P, K, 4]),
                                op=mybir.AluOpType.mult,
                            )
                            den = pool.tile([P, 4], f32, tag="den")
                            nc.vector.tensor_reduce(
                                out=den[:, :],
                                in_=e[:, :K, :].transpose([0, 2, 1]),
                                axis=mybir.AxisListType.X,
                                op=mybir.AluOpType.add,
                            )
                            nc.vector.tensor_scalar_add(
                                out=den[:, :], in0=den[:, :], scalar1=1e-30
                            )
                            rden = pool.tile([P, 4], f32, tag="rden")
                            nc.vector.reciprocal(out=rden[:, :], in_=den[:, :])
                            alpha = pool.tile([P, KMAX, 4], f16, tag="alpha")
                            nc.vector.tensor_tensor(
                                out=alpha[:, :K, :],
                                in0=e[:, :K, :],
                                in1=rden[:, :].unsqueeze(1).to_broadcast([P, K, 4]),
                                op=mybir.AluOpType.mult,
                            )
                            # ---- messages (contiguous mult + halving tree) ----
                            m = mpool.tile([P, KMAX, HID], f16, tag="m")
                            nc.vector.tensor_tensor(
                                out=m[:, :K, :].rearrange("p k (h c) -> p k h c", h=4),
                                in0=g[:, :K, :].rearrange("p k (h c) -> p k h c", h=4),
                                in1=alpha[:, :K, :]
                                .unsqueeze(3)
                                .to_broadcast([P, K, 4, PER_HEAD]),
                                op=mybir.AluOpType.mult,
                            )
                            cur = K
                            while cur > 1:
                                half = cur // 2
                                nc.vector.tensor_tensor(
                                    out=m[:, :half, :],
                                    in0=m[:, :half, :],
                                    in1=m[:, cur - half : cur, :],
                                    op=mybir.AluOpType.add,
                                )
                                cur -= half
                            nc.vector.tensor_copy(
                                out=agg_g[:, gi, :], in_=m[:, 0, :]
                            )
                        else:
                            # single-head L3 on rotated z rows: a_src3 = comp 0
                            lr3 = pool.tile([P, KMAX], f32, tag="lr3")
                            nc.vector.tensor_tensor(
                                out=lr3[:, :K],
                                in0=g[:, :K, 0],
                                in1=adst3[:, w : w + 1].to_broadcast([P, K]),
                                op=mybir.AluOpType.add,
                            )
                            nc.vector.scalar_tensor_tensor(
                                out=lr3[:, :K],
                                in0=lr3[:, :K],
                                scalar=NEG_SLOPE,
                                in1=lr3[:, :K],
                                op0=mybir.AluOpType.mult,
                                op1=mybir.AluOpType.max,
                            )
                            e3 = pool.tile([P, KMAX], f32, tag="e3")
                            nc.scalar.activation(
                                out=e3[:, :K], in_=lr3[:, :K], func=AF.Exp
                            )
                            nc.vector.tensor_tensor(
                                out=e3[:, :K],
                                in0=e3[:, :K],
                                in1=mask_sb[:, off : off + K],
                                op=mybir.AluOpType.mult,
                            )
                            den3 = pool.tile([P, 1], f32, tag="den3")
                            nc.vector.tensor_reduce(
                                out=den3[:, :],
                                in_=e3[:, :K],
                                axis=mybir.AxisListType.X,
                                op=mybir.AluOpType.add,
                            )
                            nc.vector.tensor_scalar_add(
                                out=den3[:, :], in0=den3[:, :], scalar1=1e-30
                            )
                            rden3 = pool.tile([P, 1], f32, tag="rden3")
                            nc.vector.reciprocal(out=rden3[:, :], in_=den3[:, :])
                            e16 = pool.tile([P, KMAX], f16, tag="e16")
                            nc.vector.tensor_copy(out=e16[:, :K], in_=e3[:, :K])
                            m = mpool.tile([P, KMAX, HID], f16, tag="m")
                            nc.vector.tensor_tensor(
                                out=m[:, :K, :],
                                in0=g[:, :K, :],
                                in1=e16[:, :K].unsqueeze(2).to_broadcast([P, K, HID]),
                                op=mybir.AluOpType.mult,
                            )
                            cur = K
                            while cur > 1:
                                half = cur // 2
                                nc.vector.tensor_tensor(
                                    out=m[:, :half, :],
                                    in0=m[:, :half, :],
                                    in1=m[:, cur - half : cur, :],
                                    op=mybir.AluOpType.add,
                                )
                                cur -= half
                            nc.vector.tensor_tensor(
                                out=agg_g[:, gi, :],
                                in0=m[:, 0, :],
                                in1=rden3[:, 0:1].to_broadcast([P, HID]),
                                op=mybir.AluOpType.mult,
                            )
                        off += K

                        # ---- batched tail once per group of GW windows ----
                        if gi == GW - 1:
                            w0 = w - (GW - 1)
                            if l < 2:
                                # unrotate agg' -> agg (per window, PE idle)
                                aggu = agpool.tile([P, GW, HID], f32, tag="aggu")
                                for i in range(GW):
                                    aT_p = ppt.tile([P, P], f32, tag="eluT_p")
                                    nc.tensor.transpose(
                                        out=aT_p[:],
                                        in_=agg_g[:, i, :],
                                        identity=ident[:],
                                    )
                                    aT = pool.tile([P, P], f16, tag="eluT")
                                    nc.vector.tensor_copy(out=aT[:], in_=aT_p[:])
                                    unps = pp.tile([P, HID], f32, tag="unps")
                                    nc.tensor.matmul(
                                        out=unps[:],
                                        lhsT=aT[:],
                                        rhs=unrot_sb[l][:],
                                        start=True,
                                        stop=True,
                                    )
                                    nc.vector.tensor_copy(
                                        out=aggu[:, i, :], in_=unps[:]
                                    )
                                bsb = b1_sb if l == 0 else b2_sb
                                y = pool.tile([P, GW, HID], f32, tag="y_g")
                                nc.vector.tensor_tensor(
                                    out=y[:],
                                    in0=aggu[:],
                                    in1=bsb[:].unsqueeze(1).to_broadcast([P, GW, HID]),
                                    op=mybir.AluOpType.add,
                                )
                                neg = pool.tile([P, GW, HID], f32, tag="neg_g")
                                nc.vector.tensor_scalar_min(
                                    out=neg[:], in0=y[:], scalar1=0.0
                                )
                                en = pool.tile([P, GW, HID], f32, tag="en_g")
                                nc.scalar.activation(out=en[:], in_=neg[:], func=AF.Exp)
                                elu = pool.tile([P, GW, HID], f32, tag="elu_g")
                                nc.vector.scalar_tensor_tensor(
                                    out=elu[:],
                                    in0=y[:],
                                    scalar=0.0,
                                    in1=en[:],
                                    op0=mybir.AluOpType.max,
                                    op1=mybir.AluOpType.add,
                                )
                                nc.vector.tensor_scalar_add(
                                    out=elu[:], in0=elu[:], scalar1=-1.0
                                )
                                for i in range(GW):
                                    ww = w0 + i
                                    eluT_p = ppt.tile([P, P], f32, tag="eluT_p")
                                    nc.tensor.transpose(
                                        out=eluT_p[:],
                                        in_=elu[:, i, :],
                                        identity=ident[:],
                                    )
                                    eluT = pool.tile([P, P], f16, tag="eluT")
                                    nc.vector.tensor_copy(out=eluT[:], in_=eluT_p[:])
                                    if l == 0:
                                        h2psum = pp.tile([P, TWA], f32, tag="hpsum")
                                        nc.tensor.matmul(
                                            out=h2psum[:],
                                            lhsT=eluT[:],
                                            rhs=w2aug_sb[:],
                                            start=True,
                                            stop=True,
                                        )
                                        srow = pool.tile([P, HID], f16, tag="srow")
                                        nc.vector.tensor_copy(
                                            out=srow[:], in_=h2psum[:, 0:HID]
                                        )
                                        nc.vector.tensor_copy(
                                            out=adst[1][:, ww * 4 : ww * 4 + 4],
                                            in_=h2psum[:, HID : HID + 4],
                                        )
                                        nc.sync.dma_start(
                                            out=shard[1][ww * P : (ww + 1) * P, :],
                                            in_=srow[:],
                                        )
                                    else:
                                        # table3 row: z' = elu2 @ [R3t | w3ad]
                                        z3ps = pp.tile([P, TWA], f32, tag="hpsum")
                                        nc.tensor.matmul(
                                            out=z3ps[:, 0 : HID + 1],
                                            lhsT=eluT[:],
                                            rhs=rhs3_sb[:],
                                            start=True,
                                            stop=True,
                                        )
                                        zrow = pool.tile([P, HID], f16, tag="zrow")
                                        nc.vector.tensor_copy(
                                            out=zrow[:], in_=z3ps[:, 0:HID]
                                        )
                                        nc.vector.tensor_copy(
                                            out=adst3[:, ww : ww + 1],
                                            in_=z3ps[:, HID : HID + 1],
                                        )
                                        nc.sync.dma_start(
                                            out=shard[2][ww * P : (ww + 1) * P, :],
                                            in_=zrow[:],
                                        )
                            else:
                                # L3 tail: per window W3 matmul, then batched
                                # bias+elu+log_softmax over GW windows
                                o3 = pool.tile([P, GW, N_CLASSES], f32, tag="o3g")
                                for i in range(GW):
                                    zT_p = ppt.tile([P, P], f32, tag="eluT_p")
                                    nc.tensor.transpose(
                                        out=zT_p[:],
                                        in_=agg_g[:, i, :],
                                        identity=ident[:],
                                    )
                                    zT = pool.tile([P, P], f16, tag="eluT")
                                    nc.vector.tensor_copy(out=zT[:], in_=zT_p[:])
                                    opsum = pp.tile([P, HID], f32, tag="unps")
                                    nc.tensor.matmul(
                                        out=opsum[:, 0:N_CLASSES],
                                        lhsT=zT[:],
                                        rhs=w3_sb[:],
                                        start=True,
                                        stop=True,
                                    )
                                    nc.vector.tensor_copy(
                                        out=o3[:, i, :], in_=opsum[:, 0:N_CLASSES]
                                    )
                                y0 = pool.tile([P, GW, N_CLASSES], f32, tag="y90g")
                                nc.vector.tensor_tensor(
                                    out=y0[:],
                                    in0=o3[:],
                                    in1=b3_sb[:]
                                    .unsqueeze(1)
                                    .to_broadcast([P, GW, N_CLASSES]),
                                    op=mybir.AluOpType.add,
                                )
                                n9 = pool.tile([P, GW, N_CLASSES], f32, tag="n9g")
                                nc.vector.tensor_scalar_min(
                                    out=n9[:], in0=y0[:], scalar1=0.0
                                )
                                nc.scalar.activation(out=n9[:], in_=n9[:], func=AF.Exp)
                                yb = pool.tile([P, GW, N_CLASSES], f32, tag="y9g")
                                nc.vector.scalar_tensor_tensor(
                                    out=yb[:],
                                    in0=y0[:],
                                    scalar=0.0,
                                    in1=n9[:],
                                    op0=mybir.AluOpType.max,
                                    op1=mybir.AluOpType.add,
                                )
                                nc.vector.tensor_scalar_add(
                                    out=yb[:], in0=yb[:], scalar1=-1.0
                                )
                                e9 = pool.tile([P, GW, N_CLASSES], f32, tag="e9g")
                                nc.scalar.activation(out=e9[:], in_=yb[:], func=AF.Exp)
                                s9 = pool.tile([P, GW], f32, tag="s9g")
                                nc.vector.reduce_sum(
                                    out=s9[:], in_=e9[:], axis=mybir.AxisListType.X
                                )
                                l9 = pool.tile([P, GW], f32, tag="l9g")
                                nc.scalar.activation(out=l9[:], in_=s9[:], func=AF.Ln)
                                o9 = pool.tile([P, GW, N_CLASSES], f32, tag="o9g")
                                nc.vector.tensor_tensor(
                                    out=o9[:],
                                    in0=yb[:],
                                    in1=l9[:]
                                    .unsqueeze(2)
                                    .to_broadcast([P, GW, N_CLASSES]),
                                    op=mybir.AluOpType.subtract,
                                )
                                for i in range(GW):
                                    ww = w0 + i
                                    nc.sync.dma_start(
                                        out=out_ext[ww * P : (ww + 1) * P, :],
                                        in_=o9[:, i, :],
                                    )

    return nc


# ----------------------------------------------------------------------------
# host wrapper
# ----------------------------------------------------------------------------
def _np(x):
    return np.asarray(x)


def make_in_maps(inputs):
    x = _np(inputs["x"]).astype(np.float32)
    edge_index = _np(inputs["edge_index"])
    W1 = _np(inputs["W1"]).astype(np.float32)
    as1 = _np(inputs["as1"]).astype(np.float32)
    ad1 = _np(inputs["ad1"]).astype(np.float32)
    b1 = _np(inputs["b1"]).astype(np.float32)
    W2 = _np(inputs["W2"]).astype(np.float32)
    as2 = _np(inputs["as2"]).astype(np.float32)
    ad2 = _np(inputs["ad2"]).astype(np.float32)
    b2 = _np(inputs["b2"]).astype(np.float32)
    W3 = _np(inputs["W3"]).astype(np.float32)
    as3 = _np(inputs["as3"]).astype(np.float32)
    ad3 = _np(inputs["ad3"]).astype(np.float32)
    b3 = _np(inputs["b3"]).astype(np.float32)

    A, B, Ks1, Ks2, old_of_new = prep_graph(edge_index)
    idx16, mask, S = build_idx_mask(A, B, Ks1, Ks2)

    def blockdiag(a):  # [H, C] -> [H*C, H]
        H, C = a.shape
        out = np.zeros((H * C, H), np.float32)
        for h in range(H):
            out[h * C : (h + 1) * C, h] = a[h]
        return out

    def rot_for(v):
        """Orthonormal basis with v along e1. Returns (R̃^T, U):
        h' = h @ R̃^T has h'[0] = h·v;  agg' @ U undoes the rotation."""
        C = len(v)
        n = float(np.linalg.norm(v))
        M = np.eye(C)
        M[:, 0] = v / n
        Q, _ = np.linalg.qr(M)
        if Q[:, 0] @ v < 0:
            Q[:, 0] = -Q[:, 0]
        R = Q.T.astype(np.float64)  # R @ v = n·e1
        Rt = R.copy()
        Rt[0, :] *= n
        U = R.copy()
        U[0, :] /= n
        return Rt.T, U

    def head_rot(as_mat):  # [H=4, C=32] -> (R̃_blk^T [128,128], U_blk [128,128])
        H, C = as_mat.shape
        RtT = np.zeros((H * C, H * C))
        U = np.zeros((H * C, H * C))
        for h in range(H):
            r, u = rot_for(as_mat[h])
            RtT[h * C : (h + 1) * C, h * C : (h + 1) * C] = r
            U[h * C : (h + 1) * C, h * C : (h + 1) * C] = u
        return RtT, U

    R1tT, U1 = head_rot(as1)
    R2tT, U2 = head_rot(as2)
    R3tT, U3 = rot_for(W3 @ as3[0])

    ad1b, ad2b = blockdiag(ad1), blockdiag(ad2)
    w1aug = np.concatenate([W1 @ R1tT, W1 @ ad1b], axis=1).astype(np.float16)
    w2aug = np.concatenate([W2 @ R2tT, W2 @ ad2b], axis=1).astype(np.float16)
    w3c = (U3 @ W3).astype(np.float16)
    as1r = U1.astype(np.float16)
    as2r = U2.astype(np.float16)
    w3asr = np.concatenate([R3tT, (W3 @ ad3[0])[:, None]], axis=1).astype(
        np.float16
    )

    xT = np.zeros((F_IN, NPAD), np.float16)
    real = old_of_new >= 0
    xT[:, real] = x[old_of_new[real]].T.astype(np.float16)

    b1_bc = np.broadcast_to(b1, (P, HID)).copy()
    b2_bc = np.broadcast_to(b2, (P, HID)).copy()
    b3_bc = np.broadcast_to(b3, (P, N_CLASSES)).copy()

    in_maps = []
    for c in range(NC_):
        in_maps.append(
            {
                "xT": np.ascontiguousarray(xT[:, c * NPER : (c + 1) * NPER]),
                "w1aug": w1aug,
                "w2aug": w2aug,
                "w3c": w3c,
                "as1r": as1r,
                "as2r": as2r,
                "w3asr": w3asr,
                "bias1": b1_bc,
                "bias2": b2_bc,
                "bias3": b3_bc,
                "idx": idx16[c],
                "mask": mask[c],
            }
        )
    return in_maps, Ks1, Ks2, S, old_of_new


def finalize_nc(nc):
    insert_library_loads(nc)
    split_excess_waits(nc)
    return nc


def kernel(**inputs):
    from concourse.bass_utils import run_bass_kernel_spmd

    in_maps, Ks1, Ks2, S, old_of_new = make_in_maps(inputs)
    nc = build_nc(Ks1, Ks2, S)
    finalize_nc(nc)
    res = run_bass_kernel_spmd(nc, in_maps, list(range(NC_)))
    out = np.zeros((N, N_CLASSES), np.float32)
    for c in range(NC_):
        rows = old_of_new[c * NPER : (c + 1) * NPER]
        m = rows >= 0
        out[rows[m]] = res.results[c]["out"][m]
    return out


# revision 28
# speedup vs baseline: 1.1852x; 1.0823x over previous
"""3-layer GAT (GATConv x3 + log_softmax) on 8 Trainium2 NeuronCores — V3.

Strategy (dst-sharded, edge-parallel within core). V3 replaces the per-edge-slot
indirect DMAs (994ns SWDGE overhead each, ~3000/layer) with InstDMAGatherAnt
whole-window gathers:

- Node tables are [NPAD, 128] fp16 (256B rows = h only). Per-edge a_src is
  recomputed on-device from gathered h (mult by as-blockdiag + reduce),
  which the 256B-row constraint of dma_gather forces anyway.
- dma_gather indices are int16 (max 32767), so tables are split in two
  25088-row halves; each window's edge slots are grouped [A-half | B-half].
- Indices live in SBUF wrapped [16, n/16] and replicated across all eight
  16-partition groups (each SWDGE queue's Q7 rx/tx pair reads its own
  32-partition window). 4 SWDGE queues parallelize descriptor generation.
- L3 redesigned: table3 = z = elu(out2) (so the gather row is full-width),
  aggregation happens on z, and W3 is applied post-aggregation per window.
- Messages reduced over k by a contiguous halving tree (no strided DVE pass).

Host-side prep: per-core contiguous dst ranges, degree-desc sort within core
so each 128-node window has uniform max degree; edges grouped by
(core, window, dst slot, half, k).
"""
import numpy as np

import concourse.bass as bass
import concourse.mybir as mybir
import concourse.tile as tile
from concourse.masks import make_identity

# ---- problem constants (hardcoded per contest rules) ----
N = 50000
E = 800000
F_IN = 300
HEADS = 4
PER_HEAD = 32
HID = 128
N_CLASSES = 9
NEG_SLOPE = 0.2

NC_ = 8
NPER = 6272          # nodes per core (49 * 128)
NPAD = NC_ * NPER    # 50176
HALF = NPAD // 2     # 25088 (< 32768 so int16 indices reach every row)
P = 128
NWIN = NPER // P     # 49

f32 = mybir.dt.float32
f16 = mybir.dt.float16
i16 = mybir.dt.int16
AF = mybir.ActivationFunctionType

TWA = 132  # augmented transform width: [W | W@ad_blockdiag]
import os as _os

NQ = int(_os.environ.get("GAT_NQ", "4"))  # SWDGE queues
DBG_SKIP = set(_os.environ.get("GAT_SKIP", "").split(","))
GW = 7     # windows per batched-tail group (NWIN = 7*7)


# ----------------------------------------------------------------------------
# host-side graph prep
# ----------------------------------------------------------------------------
LO_B = NPAD - 32768  # 17408: tabB = table[LO_B:], tabA = table[0:32768]
HI_A = 32768         # overlap [LO_B, HI_A) is assignable to either half


def prep_graph(edge_index):
    """Global degree-desc sort: rank r -> (core (r//128)%8, window r//1024,
    slot r%128). Overlapping A/B table slices; flexible middle-band edges are
    balanced per window to minimize K1+K2."""
    s = np.asarray(edge_index[0], dtype=np.int64)
    d = np.asarray(edge_index[1], dtype=np.int64)
    deg = np.bincount(d, minlength=N)

    rank_node = np.argsort(-deg, kind="stable")
    r = np.arange(N)
    j = r // P
    newid = (j % NC_) * NPER + (j // NC_) * P + (r % P)
    new_of_old = np.empty(N, np.int64)
    new_of_old[rank_node] = newid
    old_of_new = np.full(NPAD, -1, dtype=np.int64)
    old_of_new[newid] = rank_node

    s_new = new_of_old[s]
    d_new = new_of_old[d]
    order_all = np.lexsort((s_new, d_new))
    s_sorted = s_new[order_all]
    d_sorted = d_new[order_all]
    starts = np.searchsorted(d_sorted, np.arange(NPAD))
    ends = np.searchsorted(d_sorted, np.arange(NPAD) + 1)

    A = {}
    B = {}
    Ks1 = np.zeros(NWIN, np.int64)
    Ks2 = np.zeros(NWIN, np.int64)
    for wi in range(NWIN):
        srcs_all = {}
        mustA = np.zeros((NC_, P), np.int64)
        mustB = np.zeros((NC_, P), np.int64)
        flexn = np.zeros((NC_, P), np.int64)
        for c in range(NC_):
            base = c * NPER + wi * P
            for p in range(P):
                sr = s_sorted[starts[base + p] : ends[base + p]]
                # order: mustA, flex, mustB
                sa = sr[sr < LO_B]
                sf = sr[(sr >= LO_B) & (sr < HI_A)]
                sb = sr[sr >= HI_A]
                srcs_all[c, p] = (sa, sf, sb)
                mustA[c, p] = len(sa)
                flexn[c, p] = len(sf)
                mustB[c, p] = len(sb)
        lo = int(mustA.max())
        hi = int((mustA + flexn).max())
        bestK1, bestK2 = lo, 10**9
        for K1 in range(lo, hi + 1):
            take = np.minimum(flexn, K1 - mustA)
            K2 = int((mustB + flexn - take).max())
            if K1 + K2 < bestK1 + bestK2:
                bestK1, bestK2 = K1, K2
        Ks1[wi], Ks2[wi] = bestK1, bestK2
        for c in range(NC_):
            la, lb = [], []
            for p in range(P):
                sa, sf, sb = srcs_all[c, p]
                t = min(len(sf), bestK1 - len(sa))
                la.append(np.concatenate([sa, sf[:t]]))
                lb.append(np.concatenate([sf[t:], sb]) - LO_B)
            A[c, wi] = la
            B[c, wi] = lb
    return A, B, Ks1, Ks2, old_of_new


def wrap_idx(flat):
    """k-major flat slot indices [n] -> [128, n//16] int16, wrapped in 16
    partitions and replicated across the eight 16-partition groups."""
    n = len(flat)
    w = np.asarray(flat, np.int16).reshape(n // 16, 16).T  # [16, n//16]
    return np.tile(w, (8, 1))


def build_idx_mask(A, B, Ks1, Ks2):
    """Resident index/mask tensors. idx16: [NC_, 128, 8*S]; mask [NC_, P, S]."""
    S = int((Ks1 + Ks2).sum())
    idx16 = np.zeros((NC_, 128, 8 * S), np.int16)
    mask = np.zeros((NC_, P, S), np.float16)
    for c in range(NC_):
        col8 = 0
        col = 0
        for w in range(NWIN):
            k1, k2 = int(Ks1[w]), int(Ks2[w])
            if k1:
                arr = np.zeros((P, k1), np.int64)
                for p in range(P):
                    a = A[c, w][p]
                    arr[p, : len(a)] = a
                    mask[c, p, col : col + len(a)] = 1.0
                idx16[c, :, col8 : col8 + 8 * k1] = wrap_idx(arr.T.ravel())
                col8 += 8 * k1
                col += k1
            if k2:
                arr = np.zeros((P, k2), np.int64)
                for p in range(P):
                    b = B[c, w][p]
                    arr[p, : len(b)] = b
                    mask[c, p, col : col + len(b)] = 1.0
                idx16[c, :, col8 : col8 + 8 * k2] = wrap_idx(arr.T.ravel())
                col8 += 8 * k2
                col += k2
    return idx16, mask, S


# ----------------------------------------------------------------------------
# compile passes: library loads + walrus wait-split workaround
# ----------------------------------------------------------------------------
def insert_library_loads(nc):
    """Bacc.insert_library_loads equivalent for raw Bass: makes
    library-tracked Pool instructions (InstDMAGatherAnt) runnable."""
    from concourse.bacc import _bass_rust
    from concourse.library_config import all_libraries, standard

    inst_type_to_lib_mask = {}
    for lib in all_libraries:
        for inst_type in lib.instructions:
            inst_type_to_lib_mask[inst_type] = inst_type_to_lib_mask.get(
                inst_type, 0
            ) | (1 << lib.index)
    _bass_rust.insert_library_loads(
        nc, inst_type_to_lib_mask, len(all_libraries), standard.index
    )
    mybir.codegen_inst_isa_subclasses(nc)


def split_excess_waits(nc, max_waits=1):
    import copy

    n = 0
    for f in nc.m.functions:
        for blk in f.blocks:
            new_insts = []
            for ins in blk.instructions:
                need = (
                    ins.sync_info is not None and len(ins.sync_info.on_wait) > max_waits
                ) or (
                    isinstance(ins, mybir.InstDMACopy)
                    and getattr(ins, "queue", "") == "qPoolDynamic"
                    and ins.sync_info is not None
                    and len(ins.sync_info.on_wait) > 0
                )
                if need:
                    for w in list(ins.sync_info.on_wait):
                        noop = mybir.InstNoOp(
                            name=f"wait_split_{n}",
                            text_hint="wait_split",
                            bass_nofuse=True,
                        )
                        n += 1
                        noop.engine = ins.engine
                        si = copy.deepcopy(ins.sync_info)
                        si.on_update = type(si.on_update)()
                        si.on_wait = type(si.on_wait)([copy.deepcopy(w)])
                        noop.sync_info = si
                        new_insts.append(noop)
                    ins.sync_info.on_wait = type(ins.sync_info.on_wait)()
                new_insts.append(ins)
            if n:
                blk.instructions = new_insts
    return n


# ----------------------------------------------------------------------------
# device kernel builder
# ----------------------------------------------------------------------------
def build_nc(Ks1, Ks2, S, repeat=1, no_gather=False, nlayers=3, local_ag=False):
    """One SPMD program; per-core data differs only in tensor contents."""
    nc = bass.Bass(num_swdge_queues=NQ)
    Ks = [int(a + b) for a, b in zip(Ks1, Ks2)]
    KMAX = max(Ks)

    xT = nc.declare_dram_parameter("xT", [F_IN, NPER], f16, isOutput=False)
    w1aug = nc.declare_dram_parameter("w1aug", [F_IN, TWA], f16, isOutput=False)
    w2aug = nc.declare_dram_parameter("w2aug", [HID, TWA], f16, isOutput=False)
    w3c = nc.declare_dram_parameter("w3c", [HID, N_CLASSES], f16, isOutput=False)
    # rotation-basis matrices: L1/L2 unrotation [128,128]; L3 [R3t|w3ad] [128,129]
    as1r = nc.declare_dram_parameter("as1r", [HID, HID], f16, isOutput=False)
    as2r = nc.declare_dram_parameter("as2r", [HID, HID], f16, isOutput=False)
    w3asr = nc.declare_dram_parameter("w3asr", [HID, HID + 1], f16, isOutput=False)
    bias1 = nc.declare_dram_parameter("bias1", [P, HID], f32, isOutput=False)
    bias2 = nc.declare_dram_parameter("bias2", [P, HID], f32, isOutput=False)
    bias3 = nc.declare_dram_parameter("bias3", [P, N_CLASSES], f32, isOutput=False)
    idx_in = nc.declare_dram_parameter("idx", [128, 8 * S], i16, isOutput=False)
    mask_in = nc.declare_dram_parameter("mask", [P, S], f16, isOutput=False)
    out_ext = nc.declare_dram_parameter("out", [NPER, N_CLASSES], f32, isOutput=True)

    shard = [nc.dram_tensor(f"shard{l}", [NPER, HID], f16) for l in range(3)]
    table_sh = [
        nc.dram_tensor(f"table_sh{l}", [NPAD, HID], f16, addr_space="Shared")
        for l in range(3)
    ]

    rg = [list(range(NC_))]

    with tile.TileContext(nc) as tc:
        with (
            tc.tile_pool(name="resident", bufs=1) as rp,
            tc.tile_pool(name="sbuf", bufs=3) as pool,
            tc.tile_pool(name="gp", bufs=4) as gpool,
            tc.tile_pool(name="mp", bufs=2) as mpool,
            tc.tile_pool(name="aggp", bufs=2) as agpool,
            tc.tile_pool(name="psum", bufs=2, space="PSUM") as pp,
            tc.tile_pool(name="psum_t", bufs=2, space="PSUM") as ppt,
        ):
            # ---------- residents ----------
            idx_sb = rp.tile([128, 8 * S], i16)
            nc.sync.dma_start(out=idx_sb[:], in_=idx_in[:])
            mask_sb = rp.tile([P, S], f16)
            nc.sync.dma_start(out=mask_sb[:], in_=mask_in[:])
            ident = rp.tile([P, P], f32)
            make_identity(nc, ident[:])
            b1_sb = rp.tile([P, HID], f32)
            nc.sync.dma_start(out=b1_sb[:], in_=bias1[:])
            b2_sb = rp.tile([P, HID], f32)
            nc.sync.dma_start(out=b2_sb[:], in_=bias2[:])
            b3_sb = rp.tile([P, N_CLASSES], f32)
            nc.sync.dma_start(out=b3_sb[:], in_=bias3[:])
            w2aug_sb = rp.tile([P, TWA], f16)
            nc.sync.dma_start(out=w2aug_sb[:], in_=w2aug[:])
            w3_sb = rp.tile([P, N_CLASSES], f16)
            nc.sync.dma_start(out=w3_sb[:], in_=w3c[:])
            # unrotation matrices (agg' -> agg) for L1/L2; L3 z' row builder
            unrot_sb = [rp.tile([P, HID], f16, name=f"unrot{l}") for l in range(2)]
            nc.sync.dma_start(out=unrot_sb[0][:], in_=as1r[:])
            nc.sync.dma_start(out=unrot_sb[1][:], in_=as2r[:])
            rhs3_sb = rp.tile([P, HID + 1], f16)  # [R3t | w3ad]
            nc.sync.dma_start(out=rhs3_sb[:], in_=w3asr[:])
            w1_sb = rp.tile([P, 3 * TWA], f16)  # 3 k-chunks of w1aug
            for kc in range(3):
                kd = min(P, F_IN - kc * P)
                nc.sync.dma_start(
                    out=w1_sb[:kd, kc * TWA : kc * TWA + TWA],
                    in_=w1aug[kc * P : kc * P + kd, :],
                )
            # per-layer a_dst of own nodes; L1/L2: 4/window, L3: 1/window
            adst = [rp.tile([P, NWIN * 4], f32, name=f"adst{l}") for l in range(2)]
            adst3 = rp.tile([P, NWIN], f32)

            regs = {}

            def reg_of(v):
                if v not in regs:
                    regs[v] = nc.gpsimd.to_reg(v)
                return regs[v]

            qctr = [0]

            def next_q():
                q = qctr[0] % NQ
                qctr[0] += 1
                return q

            for _rep in range(repeat):
                # ---------- layer-1 node transform ----------
                for t in range(NWIN):
                    hpsum = pp.tile([P, TWA], f32, tag="hpsum")
                    for kc in range(3):
                        kd = min(P, F_IN - kc * P)
                        xt = pool.tile([P, P], f16, tag="xt")
                        nc.sync.dma_start(
                            out=xt[:kd, :],
                            in_=xT[kc * P : kc * P + kd, t * P : (t + 1) * P],
                        )
                        nc.tensor.matmul(
                            out=hpsum[:],
                            lhsT=xt[:kd, :],
                            rhs=w1_sb[:kd, kc * TWA : kc * TWA + TWA],
                            start=(kc == 0),
                            stop=(kc == 2),
                        )
                    hrow = pool.tile([P, HID], f16, tag="hrow")
                    nc.vector.tensor_copy(out=hrow[:], in_=hpsum[:, 0:HID])
                    nc.vector.tensor_copy(
                        out=adst[0][:, t * 4 : (t + 1) * 4], in_=hpsum[:, HID : HID + 4]
                    )
                    nc.sync.dma_start(out=shard[0][t * P : (t + 1) * P, :], in_=hrow[:])

                # ---------- per-layer: allgather + edge phase ----------
                for l in range(nlayers):
                    if local_ag:
                        # single-core stand-in for TimelineSim (no collectives)
                        nc.sync.dma_start(
                            out=table_sh[l][0:NPER, :], in_=shard[l][:]
                        )
                    else:
                        nc.gpsimd.collective_compute(
                            "AllGather",
                            mybir.AluOpType.bypass,
                            ins=[shard[l][:]],
                            outs=[table_sh[l][:]],
                            replica_groups=rg,
                        )
                    tabA = table_sh[l][0:HI_A, :]
                    tabB = table_sh[l][LO_B:NPAD, :]
                    off = 0   # mask / k-slot column offset
                    off8 = 0  # idx16 column offset
                    agg_g = None
                    for w in range(NWIN):
                        gi = w % GW
                        if gi == 0:
                            agg_g = agpool.tile([P, GW, HID], f32, tag="agg_g")
                        k1, k2 = int(Ks1[w]), int(Ks2[w])
                        K = k1 + k2
                        g = gpool.tile([P, KMAX, HID], f16, tag="g")
                        if no_gather:
                            nc.vector.memset(g[:, 0, :], 0.125)
                        else:
                            # SWDGE ring carveout limits one gather to ~64
                            # descs/engine -> chunk to <= 8 k-slots (1024 idxs)
                            CH = 8
                            for base, cnt, tab in (
                                (0, k1, tabA),
                                (k1, k2, tabB),
                            ):
                                for j in range(0, cnt, CH):
                                    ck = min(CH, cnt - j)
                                    nc.gpsimd.dma_gather(
                                        out_ap=g[:, base + j : base + j + ck, :],
                                        in_ap=tab,
                                        idxs_ap=idx_sb[:, off8 : off8 + 8 * ck],
                                        num_idxs=128 * ck,
                                        num_idxs_reg=reg_of(128 * ck),
                                        elem_size=HID,
                                        queue_num=next_q(),
                                    )
                                    off8 += 8 * ck

                        # rotated table rows: a_src per head = component h*32
                        g4 = g[:, :K, :].rearrange("p k (h c) -> p k h c", h=4)
                        if l < 2:
                            # ---- scores ----
                            lr = pool.tile([P, KMAX, 4], f32, tag="lr")
                            nc.vector.tensor_tensor(
                                out=lr[:, :K, :],
                                in0=g4[:, :, :, 0],
                                in1=adst[l][:, w * 4 : (w + 1) * 4]
                                .unsqueeze(1)
                                .to_broadcast([P, K, 4]),
                                op=mybir.AluOpType.add,
                            )
                            nc.vector.scalar_tensor_tensor(
                                out=lr[:, :K, :],
                                in0=lr[:, :K, :],
                                scalar=NEG_SLOPE,
                                in1=lr[:, :K, :],
                                op0=mybir.AluOpType.mult,
                                op1=mybir.AluOpType.max,
                            )
                            e = pool.tile([P, KMAX, 4], f32, tag="e")
                            nc.scalar.activation(
                                out=e[:, :K, :], in_=lr[:, :K, :], func=AF.Exp
                            )
                            nc.vector.tensor_tensor(
                                out=e[:, :K, :],
                                in0=e[:, :K, :],
                                in1=mask_sb[:, off : off + K]
                                .unsqueeze(2)
                                .to_broadcast([P, K, 4]),
                                op=mybir.AluOpType.mult,
                            )
                            den = pool.tile([P, 4], f32, tag="den")
                            nc.vector.tensor_reduce(
                                out=den[:, :],
                                in_=e[:, :K, :].transpose([0, 2, 1]),
                                axis=mybir.AxisListType.X,
                                op=mybir.AluOpType.add,
                            )
                            nc.vector.tensor_scalar_add(
                                out=den[:, :], in0=den[:, :], scalar1=1e-30
                            )
                            rden = pool.tile([P, 4], f32, tag="rden")
                            nc.vector.reciprocal(out=rden[:, :], in_=den[:, :])
                            alpha = pool.tile([P, KMAX, 4], f16, tag="alpha")
                            nc.vector.tensor_tensor(
                                out=alpha[:, :K, :],
                                in0=e[:, :K, :],
                                in1=rden[:, :].unsqueeze(1).to_broadcast([P, K, 4]),
                                op=mybir.AluOpType.mult,
                            )
                            # ---- messages (contiguous mult + halving tree) ----
                            m = mpool.tile([P, KMAX, HID], f16, tag="m")
                            nc.vector.tensor_tensor(
                                out=m[:, :K, :].rearrange("p k (h c) -> p k h c", h=4),
                                in0=g[:, :K, :].rearrange("p k (h c) -> p k h c", h=4),
                                in1=alpha[:, :K, :]
                                .unsqueeze(3)
                                .to_broadcast([P, K, 4, PER_HEAD]),
                                op=mybir.AluOpType.mult,
                            )
                            cur = K
                            while cur > 1:
                                half = cur // 2
                                nc.vector.tensor_tensor(
                                    out=m[:, :half, :],
                                    in0=m[:, :half, :],
                                    in1=m[:, cur - half : cur, :],
                                    op=mybir.AluOpType.add,
                                )
                                cur -= half
                            nc.vector.tensor_copy(
                                out=agg_g[:, gi, :], in_=m[:, 0, :]
                            )
                        else:
                            # single-head L3 on rotated z rows: a_src3 = comp 0
                            lr3 = pool.tile([P, KMAX], f32, tag="lr3")
                            nc.vector.tensor_tensor(
                                out=lr3[:, :K],
                                in0=g[:, :K, 0],
                                in1=adst3[:, w : w + 1].to_broadcast([P, K]),
                                op=mybir.AluOpType.add,
                            )
                            nc.vector.scalar_tensor_tensor(
                                out=lr3[:, :K],
                                in0=lr3[:, :K],
                                scalar=NEG_SLOPE,
                                in1=lr3[:, :K],
                                op0=mybir.AluOpType.mult,
                                op1=mybir.AluOpType.max,
                            )
                            e3 = pool.tile([P, KMAX], f32, tag="e3")
                            nc.scalar.activation(
                                out=e3[:, :K], in_=lr3[:, :K], func=AF.Exp
                            )
                            nc.vector.tensor_tensor(
                                out=e3[:, :K],
                                in0=e3[:, :K],
                                in1=mask_sb[:, off : off + K],
                                op=mybir.AluOpType.mult,
                            )
                            den3 = pool.tile([P, 1], f32, tag="den3")
                            nc.vector.tensor_reduce(
                                out=den3[:, :],
                                in_=e3[:, :K],
                                axis=mybir.AxisListType.X,
                                op=mybir.AluOpType.add,
                            )
                            nc.vector.tensor_scalar_add(
                                out=den3[:, :], in0=den3[:, :], scalar1=1e-30
                            )
                            rden3 = pool.tile([P, 1], f32, tag="rden3")
                            nc.vector.reciprocal(out=rden3[:, :], in_=den3[:, :])
                            e16 = pool.tile([P, KMAX], f16, tag="e16")
                            nc.vector.tensor_copy(out=e16[:, :K], in_=e3[:, :K])
                            m = mpool.tile([P, KMAX, HID], f16, tag="m")
                            nc.vector.tensor_tensor(
                                out=m[:, :K, :],
                                in0=g[:, :K, :],
                                in1=e16[:, :K].unsqueeze(2).to_broadcast([P, K, HID]),
                                op=mybir.AluOpType.mult,
                            )
                            cur = K
                            while cur > 1:
                                half = cur // 2
                                nc.vector.tensor_tensor(
                                    out=m[:, :half, :],
                                    in0=m[:, :half, :],
                                    in1=m[:, cur - half : cur, :],
                                    op=mybir.AluOpType.add,
                                )
                                cur -= half
                            nc.vector.tensor_tensor(
                                out=agg_g[:, gi, :],
                                in0=m[:, 0, :],
                                in1=rden3[:, 0:1].to_broadcast([P, HID]),
                                op=mybir.AluOpType.mult,
                            )
                        off += K

                        # ---- batched tail once per group of GW windows ----
                        if gi == GW - 1:
                            w0 = w - (GW - 1)
                            if l < 2:
                                # unrotate agg' -> agg (per window, PE idle)
                                aggu = agpool.tile([P, GW, HID], f32, tag="aggu")
                                for i in range(GW):
                                    aT_p = ppt.tile([P, P], f32, tag="eluT_p")
                                    nc.tensor.transpose(
                                        out=aT_p[:],
                                        in_=agg_g[:, i, :],
                                        identity=ident[:],
                                    )
                                    aT = pool.tile([P, P], f16, tag="eluT")
                                    nc.vector.tensor_copy(out=aT[:], in_=aT_p[:])
                                    unps = pp.tile([P, HID], f32, tag="unps")
                                    nc.tensor.matmul(
                                        out=unps[:],
                                        lhsT=aT[:],
                                        rhs=unrot_sb[l][:],
                                        start=True,
                                        stop=True,
                                    )
                                    nc.vector.tensor_copy(
                                        out=aggu[:, i, :], in_=unps[:]
                                    )
                                bsb = b1_sb if l == 0 else b2_sb
                                y = pool.tile([P, GW, HID], f32, tag="y_g")
                                nc.vector.tensor_tensor(
                                    out=y[:],
                                    in0=aggu[:],
                                    in1=bsb[:].unsqueeze(1).to_broadcast([P, GW, HID]),
                                    op=mybir.AluOpType.add,
                                )
                                neg = pool.tile([P, GW, HID], f32, tag="neg_g")
                                nc.vector.tensor_scalar_min(
                                    out=neg[:], in0=y[:], scalar1=0.0
                                )
                                en = pool.tile([P, GW, HID], f32, tag="en_g")
                                nc.scalar.activation(out=en[:], in_=neg[:], func=AF.Exp)
                                elu = pool.tile([P, GW, HID], f32, tag="elu_g")
                                nc.vector.scalar_tensor_tensor(
                                    out=elu[:],
                                    in0=y[:],
                                    scalar=0.0,
                                    in1=en[:],
                                    op0=mybir.AluOpType.max,
                                    op1=mybir.AluOpType.add,
                                )
                                nc.vector.tensor_scalar_add(
                                    out=elu[:], in0=elu[:], scalar1=-1.0
                                )
                                for i in range(GW):
                                    ww = w0 + i
                                    eluT_p = ppt.tile([P, P], f32, tag="eluT_p")
                                    nc.tensor.transpose(
                                        out=eluT_p[:],
                                        in_=elu[:, i, :],
                                        identity=ident[:],
                                    )
                                    eluT = pool.tile([P, P], f16, tag="eluT")
                                    nc.vector.tensor_copy(out=eluT[:], in_=eluT_p[:])
                                    if l == 0:
                                        h2psum = pp.tile([P, TWA], f32, tag="hpsum")
                                        nc.tensor.matmul(
                                            out=h2psum[:],
                                            lhsT=eluT[:],
                                            rhs=w2aug_sb[:],
                                            start=True,
                                            stop=True,
                                        )
                                        srow = pool.tile([P, HID], f16, tag="srow")
                                        nc.vector.tensor_copy(
                                            out=srow[:], in_=h2psum[:, 0:HID]
                                        )
                                        nc.vector.tensor_copy(
                                            out=adst[1][:, ww * 4 : ww * 4 + 4],
                                            in_=h2psum[:, HID : HID + 4],
                                        )
                                        nc.sync.dma_start(
                                            out=shard[1][ww * P : (ww + 1) * P, :],
                                            in_=srow[:],
                                        )
                                    else:
                                        # table3 row: z' = elu2 @ [R3t | w3ad]
                                        z3ps = pp.tile([P, TWA], f32, tag="hpsum")
                                        nc.tensor.matmul(
                                            out=z3ps[:, 0 : HID + 1],
                                            lhsT=eluT[:],
                                            rhs=rhs3_sb[:],
                                            start=True,
                                            stop=True,
                                        )
                                        zrow = pool.tile([P, HID], f16, tag="zrow")
                                        nc.vector.tensor_copy(
                                            out=zrow[:], in_=z3ps[:, 0:HID]
                                        )
                                        nc.vector.tensor_copy(
                                            out=adst3[:, ww : ww + 1],
                                            in_=z3ps[:, HID : HID + 1],
                                        )
                                        nc.sync.dma_start(
                                            out=shard[2][ww * P : (ww + 1) * P, :],
                                            in_=zrow[:],
                                        )
                            else:
                                # L3 tail: per window W3 matmul, then batched
                                # bias+elu+log_softmax over GW windows
                                o3 = pool.tile([P, GW, N_CLASSES], f32, tag="o3g")
                                for i in range(GW):
                                    zT_p = ppt.tile([P, P], f32, tag="eluT_p")
                                    nc.tensor.transpose(
                                        out=zT_p[:],
                                        in_=agg_g[:, i, :],
                                        identity=ident[:],
                                    )
                                    zT = pool.tile([P, P], f16, tag="eluT")
                                    nc.vector.tensor_copy(out=zT[:], in_=zT_p[:])
                                    opsum = pp.tile([P, HID], f32, tag="unps")
                                    nc.tensor.matmul(
                                        out=opsum[:, 0:N_CLASSES],
                                        lhsT=zT[:],
                                        rhs=w3_sb[:],
                                        start=True,
                                        stop=True,
                                    )
                                    nc.vector.tensor_copy(
                                        out=o3[:, i, :], in_=opsum[:, 0:N_CLASSES]
                                    )
                                y0 = pool.tile([P, GW, N_CLASSES], f32, tag="y90g")
                                nc.vector.tensor_tensor(
                                    out=y0[:],
                                    in0=o3[:],
                                    in1=b3_sb[:]
                                    .unsqueeze(1)
                                    .to_broadcast([P, GW, N_CLASSES]),
                                    op=mybir.AluOpType.add,
                                )
                                n9 = pool.tile([P, GW, N_CLASSES], f32, tag="n9g")
                                nc.vector.tensor_scalar_min(
                                    out=n9[:], in0=y0[:], scalar1=0.0
                                )
                                nc.scalar.activation(out=n9[:], in_=n9[:], func=AF.Exp)
                                yb = pool.tile([P, GW, N_CLASSES], f32, tag="y9g")
                                nc.vector.scalar_tensor_tensor(
                                    out=yb[:],
                                    in0=y0[:],
                                    scalar=0.0,
                                    in1=n9[:],
                                    op0=mybir.AluOpType.max,
                                    op1=mybir.AluOpType.add,
                                )
                                nc.vector.tensor_scalar_add(
                                    out=yb[:], in0=yb[:], scalar1=-1.0
                                )
                                e9 = pool.tile([P, GW, N_CLASSES], f32, tag="e9g")
                                nc.scalar.activation(out=e9[:], in_=yb[:], func=AF.Exp)
                                s9 = pool.tile([P, GW], f32, tag="s9g")
                                nc.vector.reduce_sum(
                                    out=s9[:], in_=e9[:], axis=mybir.AxisListType.X
                                )
                                l9 = pool.tile([P, GW], f32, tag="l9g")
                                nc.scalar.activation(out=l9[:], in_=s9[:], func=AF.Ln)
                                o9 = pool.tile([P, GW, N_CLASSES], f32, tag="o9g")
                                nc.vector.tensor_tensor(
                                    out=o9[:],
                                    in0=yb[:],
                                    in1=l9[:]
                                    .unsqueeze(2)
                                    .to_broadcast([P, GW, N_CLASSES]),
                                    op=mybir.AluOpType.subtract,
                                )
                                for i in range(GW):
                                    ww = w0 + i
                                    nc.sync.dma_start(
                                        out=out_ext[ww * P : (ww + 1) * P, :],
                                        in_=o9[:, i, :],
                                    )

    return nc


# ----------------------------------------------------------------------------
# host wrapper
# ----------------------------------------------------------------------------
def _np(x):
    return np.asarray(x)


def make_in_maps(inputs):
    x = _np(inputs["x"]).astype(np.float32)
    edge_index = _np(inputs["edge_index"])
    W1 = _np(inputs["W1"]).astype(np.float32)
    as1 = _np(inputs["as1"]).astype(np.float32)
    ad1 = _np(inputs["ad1"]).astype(np.float32)
    b1 = _np(inputs["b1"]).astype(np.float32)
    W2 = _np(inputs["W2"]).astype(np.float32)
    as2 = _np(inputs["as2"]).astype(np.float32)
    ad2 = _np(inputs["ad2"]).astype(np.float32)
    b2 = _np(inputs["b2"]).astype(np.float32)
    W3 = _np(inputs["W3"]).astype(np.float32)
    as3 = _np(inputs["as3"]).astype(np.float32)
    ad3 = _np(inputs["ad3"]).astype(np.float32)
    b3 = _np(inputs["b3"]).astype(np.float32)

    A, B, Ks1, Ks2, old_of_new = prep_graph(edge_index)
    idx16, mask, S = build_idx_mask(A, B, Ks1, Ks2)

    def blockdiag(a):  # [H, C] -> [H*C, H]
        H, C = a.shape
        out = np.zeros((H * C, H), np.float32)
        for h in range(H):
            out[h * C : (h + 1) * C, h] = a[h]
        return out

    def rot_for(v):
        """Orthonormal basis with v along e1. Returns (R̃^T, U):
        h' = h @ R̃^T has h'[0] = h·v;  agg' @ U undoes the rotation."""
        C = len(v)
        n = float(np.linalg.norm(v))
        M = np.eye(C)
        M[:, 0] = v / n
        Q, _ = np.linalg.qr(M)
        if Q[:, 0] @ v < 0:
            Q[:, 0] = -Q[:, 0]
        R = Q.T.astype(np.float64)  # R @ v = n·e1
        Rt = R.copy()
        Rt[0, :] *= n
        U = R.copy()
        U[0, :] /= n
        return Rt.T, U

    def head_rot(as_mat):  # [H=4, C=32] -> (R̃_blk^T [128,128], U_blk [128,128])
        H, C = as_mat.shape
        RtT = np.zeros((H * C, H * C))
        U = np.zeros((H * C, H * C))
        for h in range(H):
            r, u = rot_for(as_mat[h])
            RtT[h * C : (h + 1) * C, h * C : (h + 1) * C] = r
            U[h * C : (h + 1) * C, h * C : (h + 1) * C] = u
        return RtT, U

    R1tT, U1 = head_rot(as1)
    R2tT, U2 = head_rot(as2)
    R3tT, U3 = rot_for(W3 @ as3[0])

    ad1b, ad2b = blockdiag(ad1), blockdiag(ad2)
    w1aug = np.concatenate([W1 @ R1tT, W1 @ ad1b], axis=1).astype(np.float16)
    w2aug = np.concatenate([W2 @ R2tT, W2 @ ad2b], axis=1).astype(np.float16)
    w3c = (U3 @ W3).astype(np.float16)
    as1r = U1.astype(np.float16)
    as2r = U2.astype(np.float16)
    w3asr = np.concatenate([R3tT, (W3 @ ad3[0])[:, None]], axis=1).astype(
        np.float16
    )

    xT = np.zeros((F_IN, NPAD), np.float16)
    real = old_of_new >= 0
    xT[:, real] = x[old_of_new[real]].T.astype(np.float16)

    b1_bc = np.broadcast_to(b1, (P, HID)).copy()
    b2_bc = np.broadcast_to(b2, (P, HID)).copy()
    b3_bc = np.broadcast_to(b3, (P, N_CLASSES)).copy()

    in_maps = []
    for c in range(NC_):
        in_maps.append(
            {
                "xT": np.ascontiguousarray(xT[:, c * NPER : (c + 1) * NPER]),
                "w1aug": w1aug,
                "w2aug": w2aug,
                "w3c": w3c,
                "as1r": as1r,
                "as2r": as2r,
                "w3asr": w3asr,
                "bias1": b1_bc,
                "bias2": b2_bc,
                "bias3": b3_bc,
                "idx": idx16[c],
                "mask": mask[c],
            }
        )
    return in_maps, Ks1, Ks2, S, old_of_new


def finalize_nc(nc):
    insert_library_loads(nc)
    split_excess_waits(nc)
    return nc


def kernel(**inputs):
    from concourse.bass_utils import run_bass_kernel_spmd

    in_maps, Ks1, Ks2, S, old_of_new = make_in_maps(inputs)
    nc = build_nc(Ks1, Ks2, S)
    finalize_nc(nc)
    res = run_bass_kernel_spmd(nc, in_maps, list(range(NC_)))
    out = np.zeros((N, N_CLASSES), np.float32)
    for c in range(NC_):
        rows = old_of_new[c * NPER : (c + 1) * NPER]
        m = rows >= 0
        out[rows[m]] = res.results[c]["out"][m]
    return out
